# revision 30
# baseline (speedup 1.0000x reference)
"""Trainium2 Bass kernel for a single attention head with query-axis softmax.

Reference semantics (per batch b):
    k = x @ Wk; q = x @ Wq; v = x @ Wv                 # [T, H]
    wei = (q @ k^T) * E**-0.5                          # [T(query), T(key)]
    wei = where(tril, wei, -inf)                       # causal: keep s <= t
    p = softmax(wei, axis=0 over query t)              # NOTE: query axis!
    out = p @ v                                        # [T, H]

Because the softmax normalizes over the query axis t (per key column s),
out[t,h] = sum_s E[t,s] * v[s,h] / d[s] with E[t,s] = exp(wei[t,s])
(zero for s > t) and d[s] = sum_t E[t,s].  The kernel computes E^T tiles
([s on partitions, t free]) so d is a free-axis row sum (fused into the
exp instruction via accum_out), scales v rows by 1/d, and accumulates
out^T on PE.  out^T is stored as-is; the host un-transposes during the
gather (free), so no on-device layout fixup is needed.

The causal triangle mask on the diagonal block is applied ON the PE:
a 128-row matmul against the identity writes the additive -1e30 triangle
into PSUM (start of the accumulation group), and the diagonal S matmul
accumulates on top of it - no vector/gpsimd op, no cross-engine hop.

Projections: k and v are packed into one 128-partition stationary
([kT; vT] stacked, so kT and qT share partition base 0 as the S matmul
requires), q separate; two PSUM->SBUF casts per column block (kv, then
q) so the next phase's weight loads unblock early.  The v chunks are
re-transposed to natural [s, h] layout with regular 64-row matmuls
against the identity (cheaper than transpose-mode, and batched per
phase so the S-pair PSUM ring stays off vector's critical path).

Sharding: batch dim (8) across the 8 NeuronCores, weights replicated.
x is host-packed per column block ([NJ, 128, NE*CB] bf16).  A single
dma_start tops out at ~165 GB/s and per-stream rate falls as streams
are added while the aggregate rises, so each block moves as 2-3
concurrent streams spread over the idle queues (blocks 2/1 borrow the
scalar queue, which is free until the first exps), and blocks are paced
j=3..0 by tiny WAW fences on the gpsimd queue so early blocks get the
wire first.  Dummy matmuls bridge the PE from program start to the
first projection so the activity monitor never sees an idle window (14
of them: enough that the HAM clock gate opens ~+6us and stays open for
the whole matmul stream).  The out^T banks are stored as one transfer
each on the two HWDGE queues (sync/scalar): 1KB partition runs instead
of 512B halves, and gpsimd's software DGE - whose final packets drain
several us late - stays off the tail.  During the last column block the
projection PSUM pool is idle, so the 8 S-row pieces round-robin over
three 2-bank slots; each piece's bank then frees a full exp before its
reuse and the per-row exp->S serialization gap disappears.
"""

import os

import numpy as np
import ml_dtypes

import concourse.bass as bass
import concourse.tile as tile
from concourse import bacc, mybir
from concourse import bass_utils
B, T, E, H = 8, 2048, 1024, 64
P = 128                       # partitions
CB = 512                      # column block (t) width
NE = E // P                   # 8 contraction chunks for projections
NJ = T // CB                  # 4 column blocks
SCALE = float(E) ** -0.5      # note: embed**-0.5, not head_size**-0.5
MASK_NEG = -1.0e30
F32 = mybir.dt.float32
BF16 = mybir.dt.bfloat16
X = mybir.AxisListType.X
EXP = mybir.ActivationFunctionType.Exp
COPY = mybir.ActivationFunctionType.Copy

# packed weights tensor column offsets (all bf16)
WKV0 = 0                      # [P, NE*P]   chunk e: [Wk_e | Wv_e]
WQ0 = WKV0 + NE * P           # [P, NE*H]
MASK0 = WQ0 + NE * H          # [P, 4*P]    additive -1e30 triangles
ID0 = MASK0 + 4 * P           # [P, P]      identity
WALLW = ID0 + P


def _emit(tc, xb_d, wall_d, out_d):
    nc = tc.nc
    from contextlib import ExitStack

    with ExitStack() as ctx:
        singles = ctx.enter_context(tc.tile_pool(name="singles", bufs=1))
        epool = ctx.enter_context(tc.tile_pool(name="erow", bufs=9))
        dpool = ctx.enter_context(tc.tile_pool(name="dsmall", bufs=12))
        vpool = ctx.enter_context(tc.tile_pool(name="vrow", bufs=9))
        vspool = ctx.enter_context(tc.tile_pool(name="vstage", bufs=2))
        ps = ctx.enter_context(tc.tile_pool(name="ps", bufs=2, space="PSUM"))
        pproj_pool = ctx.enter_context(
            tc.tile_pool(name="pproj", bufs=1, space="PSUM")
        )
        pout = ctx.enter_context(tc.tile_pool(name="pout", bufs=1, space="PSUM"))

        # weights: k/q/v gate the first projections, so they load first;
        # masks+identity live in a separate tile fetched a bit later so
        # the x fence graph stays acyclic
        wall = singles.tile([P, MASK0], BF16, name="wall")
        wall2 = singles.tile([P, WALLW - MASK0], BF16, name="wall2")
        wkv = wall[:, WKV0 : WKV0 + NE * P]
        wq = wall[:, WQ0 : WQ0 + NE * H]
        masks = wall2[:, 0 : 4 * P]
        identb = wall2[:, 4 * P : 5 * P]

        # x blocks: two tiles per block (e-chunks 0-3 / 4-7) so the
        # projection's data deps are piece-granular, moved by 2-3
        # concurrent wire streams, with later blocks paced behind earlier
        # ones by tiny WAW fences on the gpsimd queue.
        HALF = NE * CB // 2
        xlo = {
            j: singles.tile([P, HALF], BF16, name=f"xlo{j}")
            for j in range(NJ)
        }
        xhi = {
            j: singles.tile([P, HALF], BF16, name=f"xhi{j}")
            for j in range(NJ)
        }
        junk = singles.tile([P, CB], BF16)
        nc.gpsimd.memset(junk[:], 1.0)
        # open with three streams (weights + both x3 halves): per-stream
        # rate drops with concurrency but aggregate rises, and everything
        # gates on max(weights, x3) anyway
        nc.sync.dma_start(out=wall[:], in_=wall_d[:, 0:MASK0])
        nc.scalar.dma_start(out=xlo[3][:], in_=xb_d[3][:, 0:HALF])
        nc.gpsimd.dma_start(out=xhi[3][:, 0 : HALF // 2], in_=xb_d[3][:, HALF : HALF + HALF // 2])
        nc.sync.dma_start(out=xhi[3][:, HALF // 2 :], in_=xb_d[3][:, HALF + HALF // 2 :])
        nc.scalar.dma_start(out=wall2[:], in_=wall_d[:, MASK0:])
        for j in (2, 1, 0):
            # cross-paired fences: block j's lo waits block j+1's hi and
            # vice versa, so block j starts only once block j+1 is done
            nc.gpsimd.tensor_copy(xlo[j][0:1, 0:1], xhi[j + 1][0:1, 0:1])
            nc.gpsimd.tensor_copy(xhi[j][0:1, 0:1], xlo[j + 1][0:1, 0:1])
            lo_end = HALF // 2
            nc.sync.dma_start(
                out=xlo[j][:, 0:lo_end], in_=xb_d[j][:, 0:lo_end]
            )
            if j == 0:
                # block 0's third stream: a second sync-queue issue
                # (same-queue transfers still run concurrently)
                nc.sync.dma_start(
                    out=xlo[j][:, HALF // 2 :],
                    in_=xb_d[j][:, HALF // 2 : HALF],
                )
            nc.gpsimd.dma_start(out=xhi[j][:], in_=xb_d[j][:, HALF:])
            if j in (2, 1):
                # blocks 2 and 1 are also urgent (projection drips): third
                # stream on the scalar queue, which stays idle until the
                # first exps land (block 0 would collide with them)
                nc.scalar.dma_start(
                    out=xlo[j][:, HALF // 2 :],
                    in_=xb_d[j][:, HALF // 2 : HALF],
                )


        # two dummy matmuls bridge the gap until the first projection so
        # the PE activity monitor sees a busy stream from the start (more
        # warmup wastes power-limited cycles; the projection chain itself
        # finishes the warm-up)
        pwarm = ps.tile([P, 2 * CB], F32, tag="ps", name="pwarm")
        for w in range(14):
            nc.tensor.matmul(
                pwarm[:, 0:CB],
                lhsT=junk[:, 0:P],
                rhs=junk[:],
                start=(w == 0),
                stop=(w == 13),
            )

        # per-block projected activations: [kT(0:64); vT(64:128)]; q lives
        # in one contiguous [64, T] tile (same partition base 0 as kT, as
        # the S matmul requires) so S matmuls can span two column blocks
        kqv = {
            j: singles.tile([P, CB], BF16, name=f"kqv{j}")
            for j in range(NJ)
        }
        # q partial sums live in BOTH partition halves (even chunks 0:64,
        # odd 64:128); the S stationary [kT; kT] contracts them so no zero
        # fill and no final add are needed
        q_sb = singles.tile([P, T], BF16, name="qsb")
        kdup = {
            j: singles.tile([P, CB], BF16, name=f"kdup{j}") for j in range(NJ)
        }

        # out^T accumulators packed 2 per bank: jj even rows 0:64, odd 64:128.
        # Accumulation groups on disjoint partition ranges of one bank are
        # fine on HW (per-element has_written); skip the sim's coarse check.
        pout_tiles = [
            pout.tile([P, CB], F32, tag=f"pt{a}", name=f"pt{a}") for a in range(2)
        ]
        outst = singles.tile([P, 2 * CB], BF16, name="outst")

        def pout_slice(jj, c0, c1):
            rb = H * (jj % 2)
            return pout_tiles[jj // 2][rb : rb + H, c0:c1]

        # deferred AV emission (lag behind S so PE never waits on the
        # d / v' chain): each entry = (r, j_of_row), d0, erow, vi
        pending_av = []

        def _av_one(rj, d0, erow, vi, jj):
            c = (jj - rj[1]) * CB
            lo = d0 if jj == rj[1] else 0
            nc.tensor.matmul(
                pout_slice(jj, lo, CB),
                lhsT=vi[:],
                rhs=erow[:, c + lo : c + CB],
                start=(jj == rj[1] and rj[0] == 0),
                stop=(rj[1] == 0 and rj[0] == 3),
                skip_group_check=True,
            )

        def close_bank(a):
            # stage out^T bank a PSUM->SBUF, split across vector and scalar
            # so the copy's latency halves, then store it
            half = outst[:, a * CB : (a + 1) * CB]
            nc.vector.tensor_copy(half[:, 0 : CB // 2], pout_tiles[a][:, 0 : CB // 2])
            nc.scalar.activation(
                out=half[:, CB // 2 : CB],
                in_=pout_tiles[a][:, CB // 2 : CB],
                func=COPY,
            )
            # one store per bank: 1KB partition runs (vs 512B halves) and
            # two fewer ~0.65us issue slots on the tail's critical path.
            # Both on HWDGE queues (sync/scalar) - gpsimd's software DGE
            # drains its final packets several us late.
            eng0 = nc.sync if a == 0 else nc.scalar
            eng0.dma_start(
                out=out_d[:, a * CB : (a + 1) * CB],
                in_=half[:],
            )

        def flush_av(final):
            if final:
                # all rows but the last have their v' ready well before the
                # final exp finishes, so emit them first; after vi of the
                # last row only its own four matmuls and the bank closes
                # remain on the critical path
                rows = list(pending_av)
                pending_av.clear()
                for rj, d0, erow, vi in rows[:-1]:
                    for jj in range(rj[1], NJ):
                        _av_one(rj, d0, erow, vi, jj)
                rj, d0, erow, vi = rows[-1]
                for jj in range(rj[1], 2):
                    _av_one(rj, d0, erow, vi, jj)
                close_bank(0)
                for jj in range(2, NJ):
                    _av_one(rj, d0, erow, vi, jj)
                close_bank(1)
                return
            rj, d0, erow, vi = pending_av.pop(0)
            for jj in range(rj[1], NJ):
                _av_one(rj, d0, erow, vi, jj)

        # projection matmul emission is spread through the PREVIOUS step's
        # rows so the PE instruction stream stays dense (HAM stays warm)
        def x_rhs(j, e):
            t_ = xlo[j] if e < NE // 2 else xhi[j]
            c = (e % (NE // 2)) * CB
            return t_[:, c : c + CB]

        def proj_thunks(j):
            pproj = pproj_pool.tile([P, 2 * CB], F32, tag="pp", name="pproj")

            def kv_mm(e):
                nc.tensor.matmul(
                    pproj[:, 0:CB],
                    lhsT=wkv[:, e * P : (e + 1) * P],
                    rhs=x_rhs(j, e),
                    start=(e == 0),
                    stop=(e == NE - 1),
                )

            def q_mm(e):
                # even chunks -> out partitions 0:64 (col group h0), odd ->
                # 64:128 (h64); adjacent even/odd matmuls run CONCURRENTLY
                # on the PE (same mechanism as the AV pairs), halving the
                # q-chain's array-half waste.  The S matmuls contract the
                # two partial sums against a [kT; kT] stationary.
                rb = H * (e % 2)
                nc.tensor.matmul(
                    pproj[rb : rb + H, CB : 2 * CB],
                    lhsT=wq[:, e * H : (e + 1) * H],
                    rhs=x_rhs(j, e),
                    start=(e < 2),
                    stop=(e >= NE - 2),
                    skip_group_check=True,
                )

            # lo e-chunks of both chains first, so the hi-half DMA's
            # arrival is overlapped by useful work instead of stalling
            # the kv chain midway
            thunks = []
            for e in range(NE // 2):
                thunks.append(lambda e=e: kv_mm(e))
            for e in range(NE // 2):
                thunks.append(lambda e=e: q_mm(e))
            for e in range(NE // 2, NE):
                thunks.append(lambda e=e: kv_mm(e))
            for e in range(NE // 2, NE):
                thunks.append(lambda e=e: q_mm(e))
            return pproj, thunks

        def proj_cast(j, pproj):
            # split casts (all on vector): kv first (unblocks v transposes),
            # then the kdup top half (its shift-DMA fires while the q cast
            # runs), then q (both partial-sum halves)
            nc.vector.tensor_copy(kqv[j][:], pproj[:, 0:CB])
            nc.vector.tensor_copy(kdup[j][0:H, :], pproj[0:H, 0:CB])
            # duplicate kT into partitions 64:128; queue choice matters:
            # kdup3 fires before the exp stream starts (scalar is clear),
            # kdup2..0 go to gpsimd whose queue has no later work
            eng = nc.scalar if j == 3 else nc.gpsimd
            eng.dma_start(out=kdup[j][H:P, :], in_=kdup[j][0:H, :])
            nc.vector.tensor_copy(
                q_sb[:, j * CB : (j + 1) * CB], pproj[:, CB : 2 * CB]
            )

        # --- main pipeline: column blocks in descending order --------------
        piece_ctr = [0]
        next_proj = []  # pending matmul thunks for step j-1's projections

        def drip_proj(k):
            for _ in range(min(k, len(next_proj))):
                next_proj.pop(0)()

        pproj, thunks = proj_thunks(3)
        for t in thunks:
            t()
        proj_cast(3, pproj)

        for j in reversed(range(NJ)):
            if j > 0:
                pproj_next, next_proj = proj_thunks(j - 1)

            # batch-transpose the step's four v chunks into one PSUM tile
            # and stage them in SBUF unscaled: the per-row v' scale then
            # becomes a cheap SBUF-only multiply, and the S-pair PSUM ring
            # no longer threads through vector's per-row work.  The
            # transpose is a REGULAR matmul against the identity (cost 64
            # rows each, and no transpose-mode LDWEIGHTS, which would be
            # incompatible with walrus's ldw-opt): out[s,h] =
            # sum_h' vT[h',s] I[h',h]
            vps = ps.tile([P, 2 * CB], F32, tag="ps", name="vps")
            vstage = vspool.tile([P, 4 * H], BF16, name="vstage")

            # rows i = 4j .. 4j+3 of E^T are now computable in full
            for r in range(4):
                i = 4 * j + r
                d0 = r * P  # local offset of this s-chunk within block j
                nblk = NJ - j
                npair = (nblk + 1) // 2
                erow = epool.tile([P, T], BF16)
                dparts = dpool.tile([P, 2], F32, tag="dparts")
                kT_sl = kdup[j][:, d0 : d0 + P]

                psts = []
                for pair in range(npair):
                    jj0 = j + 2 * pair
                    w = CB * min(2, NJ - jj0)  # 512 or 1024
                    if j == 0:
                        # the projection pool is idle during the last step:
                        # round-robin the 8 pieces over THREE 2-bank slots
                        # (ps.A, ps.B, pproj) so each piece's bank frees a
                        # full exp earlier than its reuse - removes the
                        # exp->S serialization gap per j=0 row
                        k = piece_ctr[0]
                        piece_ctr[0] += 1
                        if k % 3 == 2:
                            pst = pproj_pool.tile(
                                [P, 2 * CB], F32, tag="pp", name="pst0"
                            )
                        else:
                            pst = ps.tile([P, 2 * CB], F32, tag="ps")
                    else:
                        pst = ps.tile([P, 2 * CB], F32, tag="ps")
                    psts.append((pst, jj0, w))

                # additive -1e30 triangle into PSUM via the PE (identity
                # stationary), then all S matmuls back-to-back with the
                # same kT stationary (single weight load with ldw-opt)
                pst0 = psts[0][0]
                nc.tensor.matmul(
                    pst0[:, d0 : d0 + P],
                    lhsT=identb,
                    rhs=masks[:, r * P : (r + 1) * P],
                    start=True,
                    stop=False,
                )
                for pair, (pst, jj0, w) in enumerate(psts):
                    if j == 0 and pair == 1 and pending_av:
                        # the borrowed pair-1 tile serializes on the prior
                        # row's exp; keep ready AV work ahead of it in the
                        # queue so the PE never drains
                        flush_av(False)
                    t0 = jj0 * CB  # global t of this pair's first column
                    lo0 = d0 if pair == 0 else 0
                    if pair == 0:
                        # diagonal block: accumulate onto the mask first
                        nc.tensor.matmul(
                            pst[:, d0 : d0 + P],
                            lhsT=kT_sl,
                            rhs=q_sb[:, t0 + d0 : t0 + d0 + P],
                            start=False,
                            stop=True,
                        )
                        lo0 = d0 + P
                    # the rest in <=512-column pieces (the ISA caps a
                    # matmul's output at one PSUM bank)
                    c = lo0
                    while c < w:
                        ce = min(c + CB - c % CB, w)
                        nc.tensor.matmul(
                            pst[:, c:ce],
                            lhsT=kT_sl,
                            rhs=q_sb[:, t0 + c : t0 + ce],
                            start=True,
                            stop=True,
                        )
                        c = ce
                if r == 0:
                    # the v transposes are not needed until after row 0's
                    # exp, so they run BEHIND row 0's S matmuls: the phase
                    # boundary then feeds the scalar exp stream (the tail's
                    # clock) as early as possible.  Full 128-row stationary
                    # (FWL): rows 0:64 of the identity slice are zero, so
                    # the kT half contributes nothing.
                    for rr in range(4):
                        nc.tensor.matmul(
                            vps[:, rr * H : (rr + 1) * H],
                            lhsT=kqv[j][:, rr * P : (rr + 1) * P],
                            rhs=identb[:, H:P],
                            start=True,
                            stop=True,
                        )
                    nc.vector.tensor_copy(vstage[:], vps[:, 0 : 4 * H])
                drip_proj(3)

                # exp (+ d partial sums fused via accum_out), per pair.
                # For the very last row the pairs are swapped so the final
                # (tail-gating) activation is the short one
                pairs_e = list(enumerate(psts))
                if j == 0 and r == 3:
                    pairs_e.reverse()
                for pair, (pst, jj0, w) in pairs_e:
                    lo = d0 if pair == 0 else 0
                    c = 2 * CB * pair
                    nc.scalar.activation(
                        out=erow[:, c + lo : c + w],
                        in_=pst[:, lo:w],
                        func=EXP,
                        scale=SCALE,
                        accum_out=dparts[:, pair : pair + 1],
                    )

                # d = sum over the row; 1/d feeds the v' scale
                dinv = dpool.tile([P, 1], F32, tag="dinv")
                if npair > 1:
                    # plain add beats tensor_reduce (the slowest DVE op)
                    # on this 2-element sum, and it sits on the tail-
                    # critical d chain
                    dsum = dpool.tile([P, 1], F32, tag="dsum")
                    nc.vector.tensor_add(
                        dsum[:], dparts[:, 0:1], dparts[:, 1:2]
                    )
                    nc.vector.reciprocal(dinv[:], dsum[:])
                else:
                    nc.vector.reciprocal(dinv[:], dparts[:, 0:1])

                vi = vpool.tile([P, H], BF16, tag="vi", name="vi")
                nc.vector.tensor_scalar_mul(
                    vi[:], vstage[:, r * H : (r + 1) * H], dinv[:]
                )

                lag = 5 if j == 1 else (2 if j == 0 else 2)
                if len(pending_av) >= lag:
                    flush_av(False)  # AV matmuls lag behind S for overlap
                drip_proj(2)
                pending_av.append(((r, j), d0, erow, vi))

            # drain remaining next-step projection matmuls, then its cast
            drip_proj(len(next_proj))
            if j > 0:
                proj_cast(j - 1, pproj_next)

        flush_av(True)


def _enable_ldw_opt():
    """Flip walrus's --enable-ldw-opt to true for our compile: consecutive
    matmuls reusing the same stationary operand then skip the reload."""
    import concourse.bass_utils as bu

    if getattr(bu, "_ldw_opt_patched", False):
        return
    orig = bu.run_command

    def run_command_ldw(cmd, *a, **kw):
        if isinstance(cmd, list):
            cmd = [
                "--enable-ldw-opt=true" if c == "--enable-ldw-opt=false" else c
                for c in cmd
            ]
        return orig(cmd, *a, **kw)

    bu.run_command = run_command_ldw
    bu._ldw_opt_patched = True


def _build_program():
    # walrus rejects --enable-ldw-opt=true for transpose-mode LDWEIGHTS;
    # the kernel avoids transpose matmuls entirely so the opt is safe and
    # skips the stationary reload for back-to-back same-lhsT matmuls
    if os.environ.get("BASS_LDW_OPT", "0") == "1":
        _enable_ldw_opt()
    nc = bacc.Bacc("TRN2", target_bir_lowering=False, debug=False, num_devices=B)
    xb_d = nc.dram_tensor("xb", [NJ, P, NE * CB], BF16, kind="ExternalInput").ap()
    wall_d = nc.dram_tensor("wall", [P, WALLW], BF16, kind="ExternalInput").ap()
    out_d = nc.dram_tensor("out", [P, 2 * CB], BF16, kind="ExternalOutput").ap()
    with tile.TileContext(nc) as tc:
        _emit(tc, xb_d, wall_d, out_d)
    nc.compile()
    return nc


def _host_masks():
    """[128, 4*128] additive triangles: row r masks t < s within the
    diagonal 128-block (t-local f, partition p: keep f >= p)."""
    m = np.full((P, 4 * P), MASK_NEG, dtype=np.float32)
    p = np.arange(P)[:, None]
    f = np.arange(P)[None, :]
    for r in range(4):
        m[:, r * P : (r + 1) * P][f >= p] = 0.0
    return m


def _host_inputs(x, Wk, Wq, Wv):
    bf = ml_dtypes.bfloat16
    x = np.asarray(x, dtype=np.float32)
    # [B, E, T] -> block-major [B, NJ, P, NE*CB] so each block is one
    # contiguous DMA with 4KB/partition runs
    xT = np.transpose(x, (0, 2, 1)).reshape(B, NE, P, NJ, CB)
    xb = np.ascontiguousarray(xT.transpose(0, 3, 2, 1, 4)).reshape(
        B, NJ, P, NE * CB
    ).astype(bf)

    def chunks(w):  # [E, h] -> [NE, P, h]
        return np.asarray(w, np.float32).reshape(NE, P, -1)

    # wkv chunk e = [Wk_e | Wv_e] -> [P, NE*128]
    kv = np.concatenate([chunks(Wk), chunks(Wv)], axis=2)
    wkv = kv.transpose(1, 0, 2).reshape(P, NE * P)
    wqp = chunks(Wq).transpose(1, 0, 2).reshape(P, NE * H)
    wall = np.concatenate(
        [wkv, wqp, _host_masks(), np.eye(P, dtype=np.float32)], axis=1
    ).astype(bf)
    assert wall.shape == (P, WALLW)
    return [{"xb": xb[b], "wall": wall} for b in range(B)]


def _unpack_out(outT):
    """[128, 1024] out^T banks (bf16) -> [T, H] f32 natural layout."""
    outT = np.asarray(outT, dtype=np.float32)
    o = np.empty((T, H), dtype=np.float32)
    for a in range(2):
        for h2 in range(2):
            jj = 2 * a + h2
            o[jj * CB : (jj + 1) * CB, :] = outT[
                H * h2 : H * (h2 + 1), a * CB : (a + 1) * CB
            ].T
    return o


def _ensure_axon_ntff_hook():
    """The agent image's antenv lacks axon_hooks; synthesize it so
    run_bass_kernel_spmd's trace path can find the NTFF profile hook."""
    import sys
    import types

    if "antenv.axon_hooks" in sys.modules:
        return
    try:
        import antenv

        mod = types.ModuleType("antenv.axon_hooks")
        mod._hook = None

        def set_axon_ntff_profile_hook(h):
            mod._hook = h

        def get_axon_ntff_profile_hook():
            return mod._hook

        mod.set_axon_ntff_profile_hook = set_axon_ntff_profile_hook
        mod.get_axon_ntff_profile_hook = get_axon_ntff_profile_hook
        sys.modules["antenv.axon_hooks"] = mod
        antenv.axon_hooks = mod

        from trn_agent_boot.trn_boot import _ntff_profile_via_ctypes

        hook = _ntff_profile_via_ctypes("/opt/axon/libaxon_pjrt.so")
        if hook is not None:
            mod._hook = hook
    except Exception as e:  # degrade to untraced run
        print(f"NTFF hook setup failed ({e}); tracing will be skipped")


def kernel(x, Wk, Wq, Wv, _trace=False, _trace_kwargs=None):
    if _trace:
        _ensure_axon_ntff_hook()
    in_maps = _host_inputs(x, Wk, Wq, Wv)
    nc = _build_program()
    res = bass_utils.run_bass_kernel_spmd(
        nc, in_maps, list(range(B)), trace=_trace, **(_trace_kwargs or {})
    )
    out = np.stack(
        [_unpack_out(res.results[b]["out"]) for b in range(B)], axis=0
    )
    if _trace:
        kernel.last_results = res
    return out.astype(np.float32)



# revision 31
# speedup vs baseline: 1.0076x; 1.0076x over previous
"""Trainium2 Bass kernel for a single attention head with query-axis softmax.

Reference semantics (per batch b):
    k = x @ Wk; q = x @ Wq; v = x @ Wv                 # [T, H]
    wei = (q @ k^T) * E**-0.5                          # [T(query), T(key)]
    wei = where(tril, wei, -inf)                       # causal: keep s <= t
    p = softmax(wei, axis=0 over query t)              # NOTE: query axis!
    out = p @ v                                        # [T, H]

Because the softmax normalizes over the query axis t (per key column s),
out[t,h] = sum_s E[t,s] * v[s,h] / d[s] with E[t,s] = exp(wei[t,s])
(zero for s > t) and d[s] = sum_t E[t,s].  The kernel computes E^T tiles
([s on partitions, t free]) so d is a free-axis row sum (fused into the
exp instruction via accum_out), scales v rows by 1/d, and accumulates
out^T on PE.  out^T is stored as-is; the host un-transposes during the
gather (free), so no on-device layout fixup is needed.

The causal triangle mask on the diagonal block is applied ON the PE:
a 128-row matmul against the identity writes the additive -1e30 triangle
into PSUM (start of the accumulation group), and the diagonal S matmul
accumulates on top of it - no vector/gpsimd op, no cross-engine hop.

Projections: k and v are packed into one 128-partition stationary
([kT; vT] stacked, so kT and qT share partition base 0 as the S matmul
requires), q separate; two PSUM->SBUF casts per column block (kv, then
q) so the next phase's weight loads unblock early.  The v chunks are
re-transposed to natural [s, h] layout with regular 64-row matmuls
against the identity (cheaper than transpose-mode, and batched per
phase so the S-pair PSUM ring stays off vector's critical path).

Sharding: batch dim (8) across the 8 NeuronCores, weights replicated.
x is host-packed per column block ([NJ, 128, NE*CB] bf16).  A single
dma_start tops out at ~165 GB/s and per-stream rate falls as streams
are added while the aggregate rises, so each block moves as 2-3
concurrent streams spread over the idle queues (blocks 2/1 borrow the
scalar queue, which is free until the first exps), and blocks are paced
j=3..0 by tiny WAW fences on the gpsimd queue so early blocks get the
wire first.  Dummy matmuls bridge the PE from program start to the
first projection so the activity monitor never sees an idle window (14
of them: enough that the HAM clock gate opens ~+6us and stays open for
the whole matmul stream).  The out^T banks are stored as one transfer
each on the two HWDGE queues (sync/scalar): 1KB partition runs instead
of 512B halves, and gpsimd's software DGE - whose final packets drain
several us late - stays off the tail.  During the last column block the
projection PSUM pool is idle, so the 8 S-row pieces round-robin over
three 2-bank slots; each piece's bank then frees a full exp before its
reuse and the per-row exp->S serialization gap disappears.
"""

import os

import numpy as np
import ml_dtypes

import concourse.bass as bass
import concourse.tile as tile
from concourse import bacc, mybir
from concourse import bass_utils
B, T, E, H = 8, 2048, 1024, 64
P = 128                       # partitions
CB = 512                      # column block (t) width
NE = E // P                   # 8 contraction chunks for projections
NJ = T // CB                  # 4 column blocks
SCALE = float(E) ** -0.5      # note: embed**-0.5, not head_size**-0.5
MASK_NEG = -1.0e30
F32 = mybir.dt.float32
BF16 = mybir.dt.bfloat16
X = mybir.AxisListType.X
EXP = mybir.ActivationFunctionType.Exp
COPY = mybir.ActivationFunctionType.Copy

# packed weights tensor column offsets (all bf16)
WKV0 = 0                      # [P, NE*P]   chunk e: [Wk_e | Wv_e]
WQ0 = WKV0 + NE * P           # [P, NE*H]
MASK0 = WQ0 + NE * H          # [P, 4*P]    additive -1e30 triangles
ID0 = MASK0 + 4 * P           # [P, P]      identity
WALLW = ID0 + P


def _emit(tc, xb_d, wall_d, out_d):
    nc = tc.nc
    from contextlib import ExitStack

    with ExitStack() as ctx:
        singles = ctx.enter_context(tc.tile_pool(name="singles", bufs=1))
        epool = ctx.enter_context(tc.tile_pool(name="erow", bufs=9))
        dpool = ctx.enter_context(tc.tile_pool(name="dsmall", bufs=12))
        vpool = ctx.enter_context(tc.tile_pool(name="vrow", bufs=9))
        vspool = ctx.enter_context(tc.tile_pool(name="vstage", bufs=2))
        ps = ctx.enter_context(tc.tile_pool(name="ps", bufs=2, space="PSUM"))
        pproj_pool = ctx.enter_context(
            tc.tile_pool(name="pproj", bufs=1, space="PSUM")
        )
        pout = ctx.enter_context(tc.tile_pool(name="pout", bufs=1, space="PSUM"))

        # weights: k/q/v gate the first projections, so they load first;
        # masks+identity live in a separate tile fetched a bit later so
        # the x fence graph stays acyclic
        wall = singles.tile([P, MASK0], BF16, name="wall")
        wall2 = singles.tile([P, WALLW - MASK0], BF16, name="wall2")
        wkv = wall[:, WKV0 : WKV0 + NE * P]
        wq = wall[:, WQ0 : WQ0 + NE * H]
        masks = wall2[:, 0 : 4 * P]
        identb = wall2[:, 4 * P : 5 * P]

        # x blocks: two tiles per block (e-chunks 0-3 / 4-7) so the
        # projection's data deps are piece-granular, moved by 2-3
        # concurrent wire streams, with later blocks paced behind earlier
        # ones by tiny WAW fences on the gpsimd queue.
        HALF = NE * CB // 2
        xlo = {
            j: singles.tile([P, HALF], BF16, name=f"xlo{j}")
            for j in range(NJ)
        }
        xhi = {
            j: singles.tile([P, HALF], BF16, name=f"xhi{j}")
            for j in range(NJ)
        }
        junk = singles.tile([P, CB], BF16)
        nc.gpsimd.memset(junk[:], 1.0)
        # open with three streams (weights + both x3 halves): per-stream
        # rate drops with concurrency but aggregate rises, and everything
        # gates on max(weights, x3) anyway
        nc.sync.dma_start(out=wall[:], in_=wall_d[:, 0:MASK0])
        nc.scalar.dma_start(out=xlo[3][:], in_=xb_d[3][:, 0:HALF])
        nc.gpsimd.dma_start(out=xhi[3][:, 0 : HALF // 2], in_=xb_d[3][:, HALF : HALF + HALF // 2])
        nc.sync.dma_start(out=xhi[3][:, HALF // 2 :], in_=xb_d[3][:, HALF + HALF // 2 :])
        nc.scalar.dma_start(out=wall2[:], in_=wall_d[:, MASK0:])
        for j in (2, 1, 0):
            # cross-paired fences: block j's lo waits block j+1's hi and
            # vice versa, so block j starts only once block j+1 is done
            nc.gpsimd.tensor_copy(xlo[j][0:1, 0:1], xhi[j + 1][0:1, 0:1])
            nc.gpsimd.tensor_copy(xhi[j][0:1, 0:1], xlo[j + 1][0:1, 0:1])
            lo_end = HALF // 2
            nc.sync.dma_start(
                out=xlo[j][:, 0:lo_end], in_=xb_d[j][:, 0:lo_end]
            )
            if j == 0:
                # block 0's third stream: a second sync-queue issue
                # (same-queue transfers still run concurrently)
                nc.sync.dma_start(
                    out=xlo[j][:, HALF // 2 :],
                    in_=xb_d[j][:, HALF // 2 : HALF],
                )
            nc.gpsimd.dma_start(out=xhi[j][:], in_=xb_d[j][:, HALF:])
            if j in (2, 1):
                # blocks 2 and 1 are also urgent (projection drips): third
                # stream on the scalar queue, which stays idle until the
                # first exps land (block 0 would collide with them)
                nc.scalar.dma_start(
                    out=xlo[j][:, HALF // 2 :],
                    in_=xb_d[j][:, HALF // 2 : HALF],
                )


        # two dummy matmuls bridge the gap until the first projection so
        # the PE activity monitor sees a busy stream from the start (more
        # warmup wastes power-limited cycles; the projection chain itself
        # finishes the warm-up)
        pwarm = ps.tile([P, 2 * CB], F32, tag="ps", name="pwarm")
        for w in range(14):
            nc.tensor.matmul(
                pwarm[:, 0:CB],
                lhsT=junk[:, 0:P],
                rhs=junk[:],
                start=(w == 0),
                stop=(w == 13),
            )

        # per-block projected activations: [kT(0:64); vT(64:128)]; q lives
        # in one contiguous [64, T] tile (same partition base 0 as kT, as
        # the S matmul requires) so S matmuls can span two column blocks
        kqv = {
            j: singles.tile([P, CB], BF16, name=f"kqv{j}")
            for j in range(NJ)
        }
        # q partial sums live in BOTH partition halves (even chunks 0:64,
        # odd 64:128); the S stationary [kT; kT] contracts them so no zero
        # fill and no final add are needed
        q_sb = singles.tile([P, T], BF16, name="qsb")
        kdup = {
            j: singles.tile([P, CB], BF16, name=f"kdup{j}") for j in range(NJ)
        }

        # out^T accumulators packed 2 per bank: jj even rows 0:64, odd 64:128.
        # Accumulation groups on disjoint partition ranges of one bank are
        # fine on HW (per-element has_written); skip the sim's coarse check.
        pout_tiles = [
            pout.tile([P, CB], F32, tag=f"pt{a}", name=f"pt{a}") for a in range(2)
        ]
        outst = singles.tile([P, 2 * CB], BF16, name="outst")

        def pout_slice(jj, c0, c1):
            rb = H * (jj % 2)
            return pout_tiles[jj // 2][rb : rb + H, c0:c1]

        # deferred AV emission (lag behind S so PE never waits on the
        # d / v' chain): each entry = (r, j_of_row), d0, erow, vi
        pending_av = []

        def _av_one(rj, d0, erow, vi, jj):
            c = (jj - rj[1]) * CB
            lo = d0 if jj == rj[1] else 0
            nc.tensor.matmul(
                pout_slice(jj, lo, CB),
                lhsT=vi[:],
                rhs=erow[:, c + lo : c + CB],
                start=(jj == rj[1] and rj[0] == 0),
                stop=(rj[1] == 0 and rj[0] == 3),
                skip_group_check=True,
            )

        def close_bank(a):
            # stage out^T bank a PSUM->SBUF, split across vector and scalar
            # so the copy's latency halves, then store it
            half = outst[:, a * CB : (a + 1) * CB]
            nc.vector.tensor_copy(half[:, 0 : CB // 2], pout_tiles[a][:, 0 : CB // 2])
            nc.scalar.activation(
                out=half[:, CB // 2 : CB],
                in_=pout_tiles[a][:, CB // 2 : CB],
                func=COPY,
            )
            # one store per bank: 1KB partition runs (vs 512B halves) and
            # two fewer ~0.65us issue slots on the tail's critical path.
            # Both on HWDGE queues (sync/scalar) - gpsimd's software DGE
            # drains its final packets several us late.
            eng0 = nc.sync if a == 0 else nc.scalar
            eng0.dma_start(
                out=out_d[:, a * CB : (a + 1) * CB],
                in_=half[:],
            )

        def flush_av(final):
            if final:
                # all rows but the last have their v' ready well before the
                # final exp finishes, so emit them first; after vi of the
                # last row only its own four matmuls and the bank closes
                # remain on the critical path
                rows = list(pending_av)
                pending_av.clear()
                for rj, d0, erow, vi in rows[:-1]:
                    for jj in range(rj[1], NJ):
                        _av_one(rj, d0, erow, vi, jj)
                rj, d0, erow, vi = rows[-1]
                for jj in range(rj[1], 2):
                    _av_one(rj, d0, erow, vi, jj)
                close_bank(0)
                for jj in range(2, NJ):
                    _av_one(rj, d0, erow, vi, jj)
                close_bank(1)
                return
            rj, d0, erow, vi = pending_av.pop(0)
            for jj in range(rj[1], NJ):
                _av_one(rj, d0, erow, vi, jj)

        # projection matmul emission is spread through the PREVIOUS step's
        # rows so the PE instruction stream stays dense (HAM stays warm)
        def x_rhs(j, e):
            t_ = xlo[j] if e < NE // 2 else xhi[j]
            c = (e % (NE // 2)) * CB
            return t_[:, c : c + CB]

        def proj_thunks(j):
            pproj = pproj_pool.tile([P, 2 * CB], F32, tag="pp", name="pproj")

            def kv_mm(e):
                nc.tensor.matmul(
                    pproj[:, 0:CB],
                    lhsT=wkv[:, e * P : (e + 1) * P],
                    rhs=x_rhs(j, e),
                    start=(e == 0),
                    stop=(e == NE - 1),
                )

            def q_mm(e):
                # even chunks -> out partitions 0:64 (col group h0), odd ->
                # 64:128 (h64); adjacent even/odd matmuls run CONCURRENTLY
                # on the PE (same mechanism as the AV pairs), halving the
                # q-chain's array-half waste.  The S matmuls contract the
                # two partial sums against a [kT; kT] stationary.
                rb = H * (e % 2)
                nc.tensor.matmul(
                    pproj[rb : rb + H, CB : 2 * CB],
                    lhsT=wq[:, e * H : (e + 1) * H],
                    rhs=x_rhs(j, e),
                    start=(e < 2),
                    stop=(e >= NE - 2),
                    skip_group_check=True,
                )

            # lo e-chunks of both chains first, so the hi-half DMA's
            # arrival is overlapped by useful work instead of stalling
            # the kv chain midway
            thunks = []
            for e in range(NE // 2):
                thunks.append(lambda e=e: kv_mm(e))
            for e in range(NE // 2):
                thunks.append(lambda e=e: q_mm(e))
            for e in range(NE // 2, NE):
                thunks.append(lambda e=e: kv_mm(e))
            for e in range(NE // 2, NE):
                thunks.append(lambda e=e: q_mm(e))
            return pproj, thunks

        def proj_cast(j, pproj):
            # split casts (all on vector): kv first (unblocks v transposes),
            # then the kdup top half (its shift-DMA fires while the q cast
            # runs), then q (both partial-sum halves)
            nc.vector.tensor_copy(kqv[j][:], pproj[:, 0:CB])
            nc.vector.tensor_copy(kdup[j][0:H, :], pproj[0:H, 0:CB])
            # duplicate kT into partitions 64:128.  All four shifts ride
            # the scalar queue: each is emitted between block j+1's exps and
            # block j's exps, and block j's exps wait on the same cast data
            # anyway, so the issue's wait never blocks runnable work.  (On
            # gpsimd it would sit behind the x-pacing fence chain and
            # stall block j's whole S phase.)
            nc.scalar.dma_start(out=kdup[j][H:P, :], in_=kdup[j][0:H, :])
            nc.vector.tensor_copy(
                q_sb[:, j * CB : (j + 1) * CB], pproj[:, CB : 2 * CB]
            )

        # --- main pipeline: column blocks in descending order --------------
        piece_ctr = [0]
        next_proj = []  # pending matmul thunks for step j-1's projections

        def drip_proj(k):
            for _ in range(min(k, len(next_proj))):
                next_proj.pop(0)()

        pproj, thunks = proj_thunks(3)
        for t in thunks:
            t()
        proj_cast(3, pproj)

        for j in reversed(range(NJ)):
            if j > 0:
                pproj_next, next_proj = proj_thunks(j - 1)

            # batch-transpose the step's four v chunks into one PSUM tile
            # and stage them in SBUF unscaled: the per-row v' scale then
            # becomes a cheap SBUF-only multiply, and the S-pair PSUM ring
            # no longer threads through vector's per-row work.  The
            # transpose is a REGULAR matmul against the identity (cost 64
            # rows each, and no transpose-mode LDWEIGHTS, which would be
            # incompatible with walrus's ldw-opt): out[s,h] =
            # sum_h' vT[h',s] I[h',h]
            vps = ps.tile([P, 2 * CB], F32, tag="ps", name="vps")
            vstage = vspool.tile([P, 4 * H], BF16, name="vstage")

            # rows i = 4j .. 4j+3 of E^T are now computable in full
            for r in range(4):
                i = 4 * j + r
                d0 = r * P  # local offset of this s-chunk within block j
                nblk = NJ - j
                npair = (nblk + 1) // 2
                erow = epool.tile([P, T], BF16)
                dparts = dpool.tile([P, 2], F32, tag="dparts")
                kT_sl = kdup[j][:, d0 : d0 + P]

                psts = []
                for pair in range(npair):
                    jj0 = j + 2 * pair
                    w = CB * min(2, NJ - jj0)  # 512 or 1024
                    if j == 0:
                        # the projection pool is idle during the last step:
                        # round-robin the 8 pieces over THREE 2-bank slots
                        # (ps.A, ps.B, pproj) so each piece's bank frees a
                        # full exp earlier than its reuse - removes the
                        # exp->S serialization gap per j=0 row
                        k = piece_ctr[0]
                        piece_ctr[0] += 1
                        if k % 3 == 2:
                            pst = pproj_pool.tile(
                                [P, 2 * CB], F32, tag="pp", name="pst0"
                            )
                        else:
                            pst = ps.tile([P, 2 * CB], F32, tag="ps")
                    else:
                        pst = ps.tile([P, 2 * CB], F32, tag="ps")
                    psts.append((pst, jj0, w))

                # additive -1e30 triangle into PSUM via the PE (identity
                # stationary), then all S matmuls back-to-back with the
                # same kT stationary (single weight load with ldw-opt)
                pst0 = psts[0][0]
                nc.tensor.matmul(
                    pst0[:, d0 : d0 + P],
                    lhsT=identb,
                    rhs=masks[:, r * P : (r + 1) * P],
                    start=True,
                    stop=False,
                )
                for pair, (pst, jj0, w) in enumerate(psts):
                    if j == 0 and pair == 1 and pending_av:
                        # the borrowed pair-1 tile serializes on the prior
                        # row's exp; keep ready AV work ahead of it in the
                        # queue so the PE never drains
                        flush_av(False)
                    t0 = jj0 * CB  # global t of this pair's first column
                    lo0 = d0 if pair == 0 else 0
                    if pair == 0:
                        # diagonal block: accumulate onto the mask first
                        nc.tensor.matmul(
                            pst[:, d0 : d0 + P],
                            lhsT=kT_sl,
                            rhs=q_sb[:, t0 + d0 : t0 + d0 + P],
                            start=False,
                            stop=True,
                        )
                        lo0 = d0 + P
                    # the rest in <=512-column pieces (the ISA caps a
                    # matmul's output at one PSUM bank)
                    c = lo0
                    while c < w:
                        ce = min(c + CB - c % CB, w)
                        nc.tensor.matmul(
                            pst[:, c:ce],
                            lhsT=kT_sl,
                            rhs=q_sb[:, t0 + c : t0 + ce],
                            start=True,
                            stop=True,
                        )
                        c = ce
                if r == 0:
                    # the v transposes are not needed until after row 0's
                    # exp, so they run BEHIND row 0's S matmuls: the phase
                    # boundary then feeds the scalar exp stream (the tail's
                    # clock) as early as possible.  Full 128-row stationary
                    # (FWL): rows 0:64 of the identity slice are zero, so
                    # the kT half contributes nothing.
                    for rr in range(4):
                        nc.tensor.matmul(
                            vps[:, rr * H : (rr + 1) * H],
                            lhsT=kqv[j][:, rr * P : (rr + 1) * P],
                            rhs=identb[:, H:P],
                            start=True,
                            stop=True,
                        )
                    nc.vector.tensor_copy(vstage[:], vps[:, 0 : 4 * H])
                drip_proj(3)

                # exp (+ d partial sums fused via accum_out), per pair.
                # For the very last row the pairs are swapped so the final
                # (tail-gating) activation is the short one
                pairs_e = list(enumerate(psts))
                if j == 0 and r == 3:
                    pairs_e.reverse()
                for pair, (pst, jj0, w) in pairs_e:
                    lo = d0 if pair == 0 else 0
                    c = 2 * CB * pair
                    nc.scalar.activation(
                        out=erow[:, c + lo : c + w],
                        in_=pst[:, lo:w],
                        func=EXP,
                        scale=SCALE,
                        accum_out=dparts[:, pair : pair + 1],
                    )

                # d = sum over the row; 1/d feeds the v' scale
                dinv = dpool.tile([P, 1], F32, tag="dinv")
                if npair > 1:
                    # plain add beats tensor_reduce (the slowest DVE op)
                    # on this 2-element sum, and it sits on the tail-
                    # critical d chain
                    dsum = dpool.tile([P, 1], F32, tag="dsum")
                    nc.vector.tensor_add(
                        dsum[:], dparts[:, 0:1], dparts[:, 1:2]
                    )
                    nc.vector.reciprocal(dinv[:], dsum[:])
                else:
                    nc.vector.reciprocal(dinv[:], dparts[:, 0:1])

                vi = vpool.tile([P, H], BF16, tag="vi", name="vi")
                nc.vector.tensor_scalar_mul(
                    vi[:], vstage[:, r * H : (r + 1) * H], dinv[:]
                )

                lag = 5 if j == 1 else (2 if j == 0 else 2)
                if len(pending_av) >= lag:
                    flush_av(False)  # AV matmuls lag behind S for overlap
                drip_proj(2)
                pending_av.append(((r, j), d0, erow, vi))

            # drain remaining next-step projection matmuls, then its cast
            drip_proj(len(next_proj))
            if j > 0:
                proj_cast(j - 1, pproj_next)

        flush_av(True)


def _enable_ldw_opt():
    """Flip walrus's --enable-ldw-opt to true for our compile: consecutive
    matmuls reusing the same stationary operand then skip the reload."""
    import concourse.bass_utils as bu

    if getattr(bu, "_ldw_opt_patched", False):
        return
    orig = bu.run_command

    def run_command_ldw(cmd, *a, **kw):
        if isinstance(cmd, list):
            cmd = [
                "--enable-ldw-opt=true" if c == "--enable-ldw-opt=false" else c
                for c in cmd
            ]
        return orig(cmd, *a, **kw)

    bu.run_command = run_command_ldw
    bu._ldw_opt_patched = True


def _build_program():
    # walrus rejects --enable-ldw-opt=true for transpose-mode LDWEIGHTS;
    # the kernel avoids transpose matmuls entirely so the opt is safe and
    # skips the stationary reload for back-to-back same-lhsT matmuls
    if os.environ.get("BASS_LDW_OPT", "0") == "1":
        _enable_ldw_opt()
    nc = bacc.Bacc("TRN2", target_bir_lowering=False, debug=False, num_devices=B)
    xb_d = nc.dram_tensor("xb", [NJ, P, NE * CB], BF16, kind="ExternalInput").ap()
    wall_d = nc.dram_tensor("wall", [P, WALLW], BF16, kind="ExternalInput").ap()
    out_d = nc.dram_tensor("out", [P, 2 * CB], BF16, kind="ExternalOutput").ap()
    with tile.TileContext(nc) as tc:
        _emit(tc, xb_d, wall_d, out_d)
    nc.compile()
    return nc


def _host_masks():
    """[128, 4*128] additive triangles: row r masks t < s within the
    diagonal 128-block (t-local f, partition p: keep f >= p)."""
    m = np.full((P, 4 * P), MASK_NEG, dtype=np.float32)
    p = np.arange(P)[:, None]
    f = np.arange(P)[None, :]
    for r in range(4):
        m[:, r * P : (r + 1) * P][f >= p] = 0.0
    return m


def _host_inputs(x, Wk, Wq, Wv):
    bf = ml_dtypes.bfloat16
    x = np.asarray(x, dtype=np.float32)
    # [B, E, T] -> block-major [B, NJ, P, NE*CB] so each block is one
    # contiguous DMA with 4KB/partition runs
    xT = np.transpose(x, (0, 2, 1)).reshape(B, NE, P, NJ, CB)
    xb = np.ascontiguousarray(xT.transpose(0, 3, 2, 1, 4)).reshape(
        B, NJ, P, NE * CB
    ).astype(bf)

    def chunks(w):  # [E, h] -> [NE, P, h]
        return np.asarray(w, np.float32).reshape(NE, P, -1)

    # wkv chunk e = [Wk_e | Wv_e] -> [P, NE*128]
    kv = np.concatenate([chunks(Wk), chunks(Wv)], axis=2)
    wkv = kv.transpose(1, 0, 2).reshape(P, NE * P)
    wqp = chunks(Wq).transpose(1, 0, 2).reshape(P, NE * H)
    wall = np.concatenate(
        [wkv, wqp, _host_masks(), np.eye(P, dtype=np.float32)], axis=1
    ).astype(bf)
    assert wall.shape == (P, WALLW)
    return [{"xb": xb[b], "wall": wall} for b in range(B)]


def _unpack_out(outT):
    """[128, 1024] out^T banks (bf16) -> [T, H] f32 natural layout."""
    outT = np.asarray(outT, dtype=np.float32)
    o = np.empty((T, H), dtype=np.float32)
    for a in range(2):
        for h2 in range(2):
            jj = 2 * a + h2
            o[jj * CB : (jj + 1) * CB, :] = outT[
                H * h2 : H * (h2 + 1), a * CB : (a + 1) * CB
            ].T
    return o


def _ensure_axon_ntff_hook():
    """The agent image's antenv lacks axon_hooks; synthesize it so
    run_bass_kernel_spmd's trace path can find the NTFF profile hook."""
    import sys
    import types

    if "antenv.axon_hooks" in sys.modules:
        return
    try:
        import antenv

        mod = types.ModuleType("antenv.axon_hooks")
        mod._hook = None

        def set_axon_ntff_profile_hook(h):
            mod._hook = h

        def get_axon_ntff_profile_hook():
            return mod._hook

        mod.set_axon_ntff_profile_hook = set_axon_ntff_profile_hook
        mod.get_axon_ntff_profile_hook = get_axon_ntff_profile_hook
        sys.modules["antenv.axon_hooks"] = mod
        antenv.axon_hooks = mod

        from trn_agent_boot.trn_boot import _ntff_profile_via_ctypes

        hook = _ntff_profile_via_ctypes("/opt/axon/libaxon_pjrt.so")
        if hook is not None:
            mod._hook = hook
    except Exception as e:  # degrade to untraced run
        print(f"NTFF hook setup failed ({e}); tracing will be skipped")


def kernel(x, Wk, Wq, Wv, _trace=False, _trace_kwargs=None):
    if _trace:
        _ensure_axon_ntff_hook()
    in_maps = _host_inputs(x, Wk, Wq, Wv)
    nc = _build_program()
    res = bass_utils.run_bass_kernel_spmd(
        nc, in_maps, list(range(B)), trace=_trace, **(_trace_kwargs or {})
    )
    out = np.stack(
        [_unpack_out(res.results[b]["out"]) for b in range(B)], axis=0
    )
    if _trace:
        kernel.last_results = res
    return out.astype(np.float32)



# revision 32
# speedup vs baseline: 1.0584x; 1.0504x over previous
"""Trainium2 Bass kernel for a single attention head with query-axis softmax.

Reference semantics (per batch b):
    k = x @ Wk; q = x @ Wq; v = x @ Wv                 # [T, H]
    wei = (q @ k^T) * E**-0.5                          # [T(query), T(key)]
    wei = where(tril, wei, -inf)                       # causal: keep s <= t
    p = softmax(wei, axis=0 over query t)              # NOTE: query axis!
    out = p @ v                                        # [T, H]

Because the softmax normalizes over the query axis t (per key column s),
out[t,h] = sum_s E[t,s] * v[s,h] / d[s] with E[t,s] = exp(wei[t,s])
(zero for s > t) and d[s] = sum_t E[t,s].  The kernel computes E^T tiles
([s on partitions, t free]) so d is a free-axis row sum (fused into the
exp instruction via accum_out), scales v rows by 1/d, and accumulates
out^T on PE.  out^T is stored as-is; the host un-transposes during the
gather (free), so no on-device layout fixup is needed.

The causal triangle mask on the diagonal block is applied ON the PE:
a 128-row matmul against the identity writes the additive -1e30 triangle
into PSUM (start of the accumulation group), and the diagonal S matmul
accumulates on top of it - no vector/gpsimd op, no cross-engine hop.

Projections: k and v are packed into one 128-partition stationary
([kT; vT] stacked, so kT and qT share partition base 0 as the S matmul
requires), q separate; two PSUM->SBUF casts per column block (kv, then
q) so the next phase's weight loads unblock early.  The v chunks are
re-transposed to natural [s, h] layout with regular 64-row matmuls
against the identity (cheaper than transpose-mode, and batched per
phase so the S-pair PSUM ring stays off vector's critical path).

Sharding: batch dim (8) across the 8 NeuronCores, weights replicated.
x is host-packed per column block ([NJ, 128, NE*CB] bf16).  A single
dma_start tops out at ~165 GB/s and per-stream rate falls as streams
are added while the aggregate rises, so each block moves as 2-3
concurrent streams spread over the idle queues (blocks 2/1 borrow the
scalar queue, which is free until the first exps), and blocks are paced
j=3..0 by tiny WAW fences on the gpsimd queue so early blocks get the
wire first.  Dummy matmuls bridge the PE from program start to the
first projection so the activity monitor never sees an idle window (14
of them: enough that the HAM clock gate opens ~+6us and stays open for
the whole matmul stream).  The out^T banks are stored as one transfer
each on the two HWDGE queues (sync/scalar): 1KB partition runs instead
of 512B halves, and gpsimd's software DGE - whose final packets drain
several us late - stays off the tail.  During the last column block the
projection PSUM pool is idle, so the 8 S-row pieces round-robin over
three 2-bank slots; each piece's bank then frees a full exp before its
reuse and the per-row exp->S serialization gap disappears.
"""

import os

import numpy as np
import ml_dtypes

import concourse.bass as bass
import concourse.tile as tile
from concourse import bacc, mybir
from concourse import bass_utils
B, T, E, H = 8, 2048, 1024, 64
P = 128                       # partitions
CB = 512                      # column block (t) width
NE = E // P                   # 8 contraction chunks for projections
NJ = T // CB                  # 4 column blocks
SCALE = float(E) ** -0.5      # note: embed**-0.5, not head_size**-0.5
MASK_NEG = -1.0e30
F32 = mybir.dt.float32
BF16 = mybir.dt.bfloat16
X = mybir.AxisListType.X
EXP = mybir.ActivationFunctionType.Exp
COPY = mybir.ActivationFunctionType.Copy

# packed weights tensor column offsets (all bf16)
WKV0 = 0                      # [P, NE*P]   chunk e: [Wk_e | Wv_e]
WQ0 = WKV0 + NE * P           # [P, NE*H]
MASK0 = WQ0 + NE * H          # [P, 4*P]    additive -1e30 triangles
ID0 = MASK0 + 4 * P           # [P, P]      identity
WALLW = ID0 + P


def _emit(tc, xb_d, wall_d, out_d):
    nc = tc.nc
    from contextlib import ExitStack

    with ExitStack() as ctx:
        singles = ctx.enter_context(tc.tile_pool(name="singles", bufs=1))
        epool = ctx.enter_context(tc.tile_pool(name="erow", bufs=9))
        dpool = ctx.enter_context(tc.tile_pool(name="dsmall", bufs=12))
        vpool = ctx.enter_context(tc.tile_pool(name="vrow", bufs=9))
        vspool = ctx.enter_context(tc.tile_pool(name="vstage", bufs=2))
        ps = ctx.enter_context(tc.tile_pool(name="ps", bufs=2, space="PSUM"))
        pproj_pool = ctx.enter_context(
            tc.tile_pool(name="pproj", bufs=1, space="PSUM")
        )
        pout = ctx.enter_context(tc.tile_pool(name="pout", bufs=1, space="PSUM"))

        # weights: k/q/v gate the first projections, so they load first;
        # masks+identity live in a separate tile fetched a bit later so
        # the x fence graph stays acyclic
        wall = singles.tile([P, MASK0], BF16, name="wall")
        wall2 = singles.tile([P, WALLW - MASK0], BF16, name="wall2")
        wkv = wall[:, WKV0 : WKV0 + NE * P]
        wq = wall[:, WQ0 : WQ0 + NE * H]
        masks = wall2[:, 0 : 4 * P]
        identb = wall2[:, 4 * P : 5 * P]

        # x blocks: two tiles per block (e-chunks 0-3 / 4-7) so the
        # projection's data deps are piece-granular, moved by 2-3
        # concurrent wire streams, with later blocks paced behind earlier
        # ones by tiny WAW fences on the gpsimd queue.
        HALF = NE * CB // 2
        xlo = {
            j: singles.tile([P, HALF], BF16, name=f"xlo{j}")
            for j in range(NJ)
        }
        xhi = {
            j: singles.tile([P, HALF], BF16, name=f"xhi{j}")
            for j in range(NJ)
        }
        junk = singles.tile([P, CB], BF16)
        nc.gpsimd.memset(junk[:], 1.0)
        # open with three streams (weights + both x3 halves): per-stream
        # rate drops with concurrency but aggregate rises, and everything
        # gates on max(weights, x3) anyway
        nc.sync.dma_start(out=wall[:], in_=wall_d[:, 0:MASK0])
        nc.scalar.dma_start(out=xlo[3][:], in_=xb_d[3][:, 0:HALF])
        nc.gpsimd.dma_start(out=xhi[3][:, 0 : HALF // 2], in_=xb_d[3][:, HALF : HALF + HALF // 2])
        nc.sync.dma_start(out=xhi[3][:, HALF // 2 :], in_=xb_d[3][:, HALF + HALF // 2 :])
        nc.scalar.dma_start(out=wall2[:], in_=wall_d[:, MASK0:])
        for j in (2, 1, 0):
            # cross-paired fences: block j's lo waits block j+1's hi and
            # vice versa, so block j starts only once block j+1 is done
            nc.gpsimd.tensor_copy(xlo[j][0:1, 0:1], xhi[j + 1][0:1, 0:1])
            nc.gpsimd.tensor_copy(xhi[j][0:1, 0:1], xlo[j + 1][0:1, 0:1])
            lo_end = HALF // 2
            nc.sync.dma_start(
                out=xlo[j][:, 0:lo_end], in_=xb_d[j][:, 0:lo_end]
            )
            if j == 0:
                # block 0's third stream: a second sync-queue issue
                # (same-queue transfers still run concurrently)
                nc.sync.dma_start(
                    out=xlo[j][:, HALF // 2 :],
                    in_=xb_d[j][:, HALF // 2 : HALF],
                )
            nc.gpsimd.dma_start(out=xhi[j][:], in_=xb_d[j][:, HALF:])
            if j in (2, 1):
                # blocks 2 and 1 are also urgent (projection drips): third
                # stream on the scalar queue, which stays idle until the
                # first exps land (block 0 would collide with them)
                nc.scalar.dma_start(
                    out=xlo[j][:, HALF // 2 :],
                    in_=xb_d[j][:, HALF // 2 : HALF],
                )


        # two dummy matmuls bridge the gap until the first projection so
        # the PE activity monitor sees a busy stream from the start (more
        # warmup wastes power-limited cycles; the projection chain itself
        # finishes the warm-up)
        pwarm = ps.tile([P, 2 * CB], F32, tag="ps", name="pwarm")
        for w in range(14):
            nc.tensor.matmul(
                pwarm[:, 0:CB],
                lhsT=junk[:, 0:P],
                rhs=junk[:],
                start=(w == 0),
                stop=(w == 13),
            )

        # per-block projected activations: [kT(0:64); vT(64:128)]; q lives
        # in one contiguous [64, T] tile (same partition base 0 as kT, as
        # the S matmul requires) so S matmuls can span two column blocks
        kqv = {
            j: singles.tile([P, CB], BF16, name=f"kqv{j}")
            for j in range(NJ)
        }
        # q partial sums live in BOTH partition halves (even chunks 0:64,
        # odd 64:128); the S stationary [kT; kT] contracts them so no zero
        # fill and no final add are needed
        q_sb = singles.tile([P, T], BF16, name="qsb")
        kdup = {
            j: singles.tile([P, CB], BF16, name=f"kdup{j}") for j in range(NJ)
        }

        # out^T accumulators packed 2 per bank: jj even rows 0:64, odd 64:128.
        # Accumulation groups on disjoint partition ranges of one bank are
        # fine on HW (per-element has_written); skip the sim's coarse check.
        pout_tiles = [
            pout.tile([P, CB], F32, tag=f"pt{a}", name=f"pt{a}") for a in range(2)
        ]
        outst = singles.tile([P, 2 * CB], BF16, name="outst")

        def pout_slice(jj, c0, c1):
            rb = H * (jj % 2)
            return pout_tiles[jj // 2][rb : rb + H, c0:c1]

        # deferred AV emission (lag behind S so PE never waits on the
        # d / v' chain): each entry = (r, j_of_row), d0, erow, vi
        pending_av = []

        def _av_one(rj, d0, erow, vi, jj):
            c = (jj - rj[1]) * CB
            lo = d0 if jj == rj[1] else 0
            nc.tensor.matmul(
                pout_slice(jj, lo, CB),
                lhsT=vi[:],
                rhs=erow[:, c + lo : c + CB],
                start=(jj == rj[1] and rj[0] == 0),
                stop=(rj[1] == 0 and rj[0] == 3),
                skip_group_check=True,
            )

        def close_bank(a):
            # stage out^T bank a PSUM->SBUF, split across vector and scalar
            # so the copy's latency halves, then store it
            half = outst[:, a * CB : (a + 1) * CB]
            nc.vector.tensor_copy(half[:, 0 : CB // 2], pout_tiles[a][:, 0 : CB // 2])
            nc.scalar.activation(
                out=half[:, CB // 2 : CB],
                in_=pout_tiles[a][:, CB // 2 : CB],
                func=COPY,
            )
            # one store per bank: 1KB partition runs (vs 512B halves) and
            # two fewer ~0.65us issue slots on the tail's critical path.
            # Both on HWDGE queues (sync/scalar) - gpsimd's software DGE
            # drains its final packets several us late.
            eng0 = nc.sync if a == 0 else nc.scalar
            eng0.dma_start(
                out=out_d[:, a * CB : (a + 1) * CB],
                in_=half[:],
            )

        def flush_av(final):
            if final:
                # all rows but the last have their v' ready well before the
                # final exp finishes, so emit them first; after vi of the
                # last row only its own four matmuls and the bank closes
                # remain on the critical path
                rows = list(pending_av)
                pending_av.clear()
                for rj, d0, erow, vi in rows[:-1]:
                    for jj in range(rj[1], NJ):
                        _av_one(rj, d0, erow, vi, jj)
                rj, d0, erow, vi = rows[-1]
                for jj in range(rj[1], 2):
                    _av_one(rj, d0, erow, vi, jj)
                close_bank(0)
                for jj in range(2, NJ):
                    _av_one(rj, d0, erow, vi, jj)
                close_bank(1)
                return
            rj, d0, erow, vi = pending_av.pop(0)
            for jj in range(rj[1], NJ):
                _av_one(rj, d0, erow, vi, jj)

        # projection matmul emission is spread through the PREVIOUS step's
        # rows so the PE instruction stream stays dense (HAM stays warm)
        def x_rhs(j, e):
            t_ = xlo[j] if e < NE // 2 else xhi[j]
            c = (e % (NE // 2)) * CB
            return t_[:, c : c + CB]

        def proj_thunks(j):
            pproj = pproj_pool.tile([P, 2 * CB], F32, tag="pp", name="pproj")

            def kv_mm(e):
                nc.tensor.matmul(
                    pproj[:, 0:CB],
                    lhsT=wkv[:, e * P : (e + 1) * P],
                    rhs=x_rhs(j, e),
                    start=(e == 0),
                    stop=(e == NE - 1),
                )

            def q_mm(e):
                # even chunks -> out partitions 0:64 (col group h0), odd ->
                # 64:128 (h64); adjacent even/odd matmuls run CONCURRENTLY
                # on the PE (same mechanism as the AV pairs), halving the
                # q-chain's array-half waste.  The S matmuls contract the
                # two partial sums against a [kT; kT] stationary.
                rb = H * (e % 2)
                nc.tensor.matmul(
                    pproj[rb : rb + H, CB : 2 * CB],
                    lhsT=wq[:, e * H : (e + 1) * H],
                    rhs=x_rhs(j, e),
                    start=(e < 2),
                    stop=(e >= NE - 2),
                    skip_group_check=True,
                )

            def cast_k():
                # kT casts + the [kT;kT] duplication DMA fire while the q
                # chain is still on the PE, hiding the shift latency
                nc.vector.tensor_copy(kqv[j][:], pproj[:, 0:CB])
                nc.vector.tensor_copy(kdup[j][0:H, :], pproj[0:H, 0:CB])
                nc.scalar.dma_start(out=kdup[j][H:P, :], in_=kdup[j][0:H, :])

            thunks = []
            if j == 3:
                # front block: interleave with the lo/hi DMA arrival
                for e in range(NE // 2):
                    thunks.append(lambda e=e: kv_mm(e))
                for e in range(NE // 2):
                    thunks.append(lambda e=e: q_mm(e))
                for e in range(NE // 2, NE):
                    thunks.append(lambda e=e: kv_mm(e))
                for e in range(NE // 2, NE):
                    thunks.append(lambda e=e: q_mm(e))
            else:
                # x is fully resident by drip time: kv chain first so the
                # kT casts + kdup shift overlap the q chain
                for e in range(NE):
                    thunks.append(lambda e=e: kv_mm(e))
                thunks.append(cast_k)
                for e in range(NE):
                    thunks.append(lambda e=e: q_mm(e))
            return pproj, thunks

        def proj_cast(j, pproj):
            if j == 3:
                nc.vector.tensor_copy(kqv[j][:], pproj[:, 0:CB])
                nc.vector.tensor_copy(kdup[j][0:H, :], pproj[0:H, 0:CB])
                nc.scalar.dma_start(out=kdup[j][H:P, :], in_=kdup[j][0:H, :])
            nc.vector.tensor_copy(
                q_sb[:, j * CB : (j + 1) * CB], pproj[:, CB : 2 * CB]
            )

        # --- main pipeline: column blocks in descending order --------------
        piece_ctr = [0]
        next_proj = []  # pending matmul thunks for step j-1's projections

        def drip_proj(k):
            for _ in range(min(k, len(next_proj))):
                next_proj.pop(0)()

        pproj, thunks = proj_thunks(3)
        for t in thunks:
            t()
        proj_cast(3, pproj)

        for j in reversed(range(NJ)):
            if j > 0:
                pproj_next, next_proj = proj_thunks(j - 1)

            # batch-transpose the step's four v chunks into one PSUM tile
            # and stage them in SBUF unscaled: the per-row v' scale then
            # becomes a cheap SBUF-only multiply, and the S-pair PSUM ring
            # no longer threads through vector's per-row work.  The
            # transpose is a REGULAR matmul against the identity (cost 64
            # rows each, and no transpose-mode LDWEIGHTS, which would be
            # incompatible with walrus's ldw-opt): out[s,h] =
            # sum_h' vT[h',s] I[h',h]
            vps = ps.tile([P, 2 * CB], F32, tag="ps", name="vps")
            vstage = vspool.tile([P, 4 * H], BF16, name="vstage")

            # rows i = 4j .. 4j+3 of E^T are now computable in full
            for r in range(4):
                i = 4 * j + r
                d0 = r * P  # local offset of this s-chunk within block j
                nblk = NJ - j
                npair = (nblk + 1) // 2
                erow = epool.tile([P, T], BF16)
                dparts = dpool.tile([P, 2], F32, tag="dparts")
                kT_sl = kdup[j][:, d0 : d0 + P]

                psts = []
                for pair in range(npair):
                    jj0 = j + 2 * pair
                    w = CB * min(2, NJ - jj0)  # 512 or 1024
                    if j == 0:
                        # the projection pool is idle during the last step:
                        # round-robin the 8 pieces over THREE 2-bank slots
                        # (ps.A, ps.B, pproj) so each piece's bank frees a
                        # full exp earlier than its reuse - removes the
                        # exp->S serialization gap per j=0 row
                        k = piece_ctr[0]
                        piece_ctr[0] += 1
                        if k % 3 == 2:
                            pst = pproj_pool.tile(
                                [P, 2 * CB], F32, tag="pp", name="pst0"
                            )
                        else:
                            pst = ps.tile([P, 2 * CB], F32, tag="ps")
                    else:
                        pst = ps.tile([P, 2 * CB], F32, tag="ps")
                    psts.append((pst, jj0, w))

                # additive -1e30 triangle into PSUM via the PE (identity
                # stationary), then all S matmuls back-to-back with the
                # same kT stationary (single weight load with ldw-opt)
                pst0 = psts[0][0]
                nc.tensor.matmul(
                    pst0[:, d0 : d0 + P],
                    lhsT=identb,
                    rhs=masks[:, r * P : (r + 1) * P],
                    start=True,
                    stop=False,
                )
                for pair, (pst, jj0, w) in enumerate(psts):
                    if j == 0 and pair == 1 and pending_av:
                        # the borrowed pair-1 tile serializes on the prior
                        # row's exp; keep ready AV work ahead of it in the
                        # queue so the PE never drains
                        flush_av(False)
                    t0 = jj0 * CB  # global t of this pair's first column
                    lo0 = d0 if pair == 0 else 0
                    if pair == 0:
                        # diagonal block: accumulate onto the mask first
                        nc.tensor.matmul(
                            pst[:, d0 : d0 + P],
                            lhsT=kT_sl,
                            rhs=q_sb[:, t0 + d0 : t0 + d0 + P],
                            start=False,
                            stop=True,
                        )
                        lo0 = d0 + P
                    # the rest in <=512-column pieces (the ISA caps a
                    # matmul's output at one PSUM bank)
                    c = lo0
                    while c < w:
                        ce = min(c + CB - c % CB, w)
                        nc.tensor.matmul(
                            pst[:, c:ce],
                            lhsT=kT_sl,
                            rhs=q_sb[:, t0 + c : t0 + ce],
                            start=True,
                            stop=True,
                        )
                        c = ce
                if r == 0:
                    # the v transposes are not needed until after row 0's
                    # exp, so they run BEHIND row 0's S matmuls: the phase
                    # boundary then feeds the scalar exp stream (the tail's
                    # clock) as early as possible.  Full 128-row stationary
                    # (FWL): rows 0:64 of the identity slice are zero, so
                    # the kT half contributes nothing.
                    for rr in range(4):
                        nc.tensor.matmul(
                            vps[:, rr * H : (rr + 1) * H],
                            lhsT=kqv[j][:, rr * P : (rr + 1) * P],
                            rhs=identb[:, H:P],
                            start=True,
                            stop=True,
                        )
                    nc.vector.tensor_copy(vstage[:], vps[:, 0 : 4 * H])
                drip_proj(3)

                # exp (+ d partial sums fused via accum_out), per pair.
                # For the very last row the pairs are swapped so the final
                # (tail-gating) activation is the short one
                pairs_e = list(enumerate(psts))
                if j == 0 and r == 3:
                    pairs_e.reverse()
                for pair, (pst, jj0, w) in pairs_e:
                    lo = d0 if pair == 0 else 0
                    c = 2 * CB * pair
                    nc.scalar.activation(
                        out=erow[:, c + lo : c + w],
                        in_=pst[:, lo:w],
                        func=EXP,
                        scale=SCALE,
                        accum_out=dparts[:, pair : pair + 1],
                    )

                # d = sum over the row; 1/d feeds the v' scale
                dinv = dpool.tile([P, 1], F32, tag="dinv")
                if npair > 1:
                    # plain add beats tensor_reduce (the slowest DVE op)
                    # on this 2-element sum, and it sits on the tail-
                    # critical d chain
                    dsum = dpool.tile([P, 1], F32, tag="dsum")
                    nc.vector.tensor_add(
                        dsum[:], dparts[:, 0:1], dparts[:, 1:2]
                    )
                    nc.vector.reciprocal(dinv[:], dsum[:])
                else:
                    nc.vector.reciprocal(dinv[:], dparts[:, 0:1])

                vi = vpool.tile([P, H], BF16, tag="vi", name="vi")
                nc.vector.tensor_scalar_mul(
                    vi[:], vstage[:, r * H : (r + 1) * H], dinv[:]
                )

                lag = 5 if j == 1 else (2 if j == 0 else 2)
                if len(pending_av) >= lag:
                    flush_av(False)  # AV matmuls lag behind S for overlap
                drip_proj(2)
                pending_av.append(((r, j), d0, erow, vi))

            # drain remaining next-step projection matmuls, then its cast
            drip_proj(len(next_proj))
            if j > 0:
                proj_cast(j - 1, pproj_next)

        flush_av(True)


def _enable_ldw_opt():
    """Flip walrus's --enable-ldw-opt to true for our compile: consecutive
    matmuls reusing the same stationary operand then skip the reload."""
    import concourse.bass_utils as bu

    if getattr(bu, "_ldw_opt_patched", False):
        return
    orig = bu.run_command

    def run_command_ldw(cmd, *a, **kw):
        if isinstance(cmd, list):
            cmd = [
                "--enable-ldw-opt=true" if c == "--enable-ldw-opt=false" else c
                for c in cmd
            ]
        return orig(cmd, *a, **kw)

    bu.run_command = run_command_ldw
    bu._ldw_opt_patched = True


def _build_program():
    # walrus rejects --enable-ldw-opt=true for transpose-mode LDWEIGHTS;
    # the kernel avoids transpose matmuls entirely so the opt is safe and
    # skips the stationary reload for back-to-back same-lhsT matmuls
    if os.environ.get("BASS_LDW_OPT", "0") == "1":
        _enable_ldw_opt()
    nc = bacc.Bacc("TRN2", target_bir_lowering=False, debug=False, num_devices=B)
    xb_d = nc.dram_tensor("xb", [NJ, P, NE * CB], BF16, kind="ExternalInput").ap()
    wall_d = nc.dram_tensor("wall", [P, WALLW], BF16, kind="ExternalInput").ap()
    out_d = nc.dram_tensor("out", [P, 2 * CB], BF16, kind="ExternalOutput").ap()
    with tile.TileContext(nc) as tc:
        _emit(tc, xb_d, wall_d, out_d)
    nc.compile()
    return nc


def _host_masks():
    """[128, 4*128] additive triangles: row r masks t < s within the
    diagonal 128-block (t-local f, partition p: keep f >= p)."""
    m = np.full((P, 4 * P), MASK_NEG, dtype=np.float32)
    p = np.arange(P)[:, None]
    f = np.arange(P)[None, :]
    for r in range(4):
        m[:, r * P : (r + 1) * P][f >= p] = 0.0
    return m


def _host_inputs(x, Wk, Wq, Wv):
    bf = ml_dtypes.bfloat16
    x = np.asarray(x, dtype=np.float32)
    # [B, E, T] -> block-major [B, NJ, P, NE*CB] so each block is one
    # contiguous DMA with 4KB/partition runs
    xT = np.transpose(x, (0, 2, 1)).reshape(B, NE, P, NJ, CB)
    xb = np.ascontiguousarray(xT.transpose(0, 3, 2, 1, 4)).reshape(
        B, NJ, P, NE * CB
    ).astype(bf)

    def chunks(w):  # [E, h] -> [NE, P, h]
        return np.asarray(w, np.float32).reshape(NE, P, -1)

    # wkv chunk e = [Wk_e | Wv_e] -> [P, NE*128]
    kv = np.concatenate([chunks(Wk), chunks(Wv)], axis=2)
    wkv = kv.transpose(1, 0, 2).reshape(P, NE * P)
    wqp = chunks(Wq).transpose(1, 0, 2).reshape(P, NE * H)
    wall = np.concatenate(
        [wkv, wqp, _host_masks(), np.eye(P, dtype=np.float32)], axis=1
    ).astype(bf)
    assert wall.shape == (P, WALLW)
    return [{"xb": xb[b], "wall": wall} for b in range(B)]


def _unpack_out(outT):
    """[128, 1024] out^T banks (bf16) -> [T, H] f32 natural layout."""
    outT = np.asarray(outT, dtype=np.float32)
    o = np.empty((T, H), dtype=np.float32)
    for a in range(2):
        for h2 in range(2):
            jj = 2 * a + h2
            o[jj * CB : (jj + 1) * CB, :] = outT[
                H * h2 : H * (h2 + 1), a * CB : (a + 1) * CB
            ].T
    return o


def _ensure_axon_ntff_hook():
    """The agent image's antenv lacks axon_hooks; synthesize it so
    run_bass_kernel_spmd's trace path can find the NTFF profile hook."""
    import sys
    import types

    if "antenv.axon_hooks" in sys.modules:
        return
    try:
        import antenv

        mod = types.ModuleType("antenv.axon_hooks")
        mod._hook = None

        def set_axon_ntff_profile_hook(h):
            mod._hook = h

        def get_axon_ntff_profile_hook():
            return mod._hook

        mod.set_axon_ntff_profile_hook = set_axon_ntff_profile_hook
        mod.get_axon_ntff_profile_hook = get_axon_ntff_profile_hook
        sys.modules["antenv.axon_hooks"] = mod
        antenv.axon_hooks = mod

        from trn_agent_boot.trn_boot import _ntff_profile_via_ctypes

        hook = _ntff_profile_via_ctypes("/opt/axon/libaxon_pjrt.so")
        if hook is not None:
            mod._hook = hook
    except Exception as e:  # degrade to untraced run
        print(f"NTFF hook setup failed ({e}); tracing will be skipped")


def kernel(x, Wk, Wq, Wv, _trace=False, _trace_kwargs=None):
    if _trace:
        _ensure_axon_ntff_hook()
    in_maps = _host_inputs(x, Wk, Wq, Wv)
    nc = _build_program()
    res = bass_utils.run_bass_kernel_spmd(
        nc, in_maps, list(range(B)), trace=_trace, **(_trace_kwargs or {})
    )
    out = np.stack(
        [_unpack_out(res.results[b]["out"]) for b in range(B)], axis=0
    )
    if _trace:
        kernel.last_results = res
    return out.astype(np.float32)



# revision 33
# speedup vs baseline: 1.1224x; 1.0605x over previous
"""Trainium2 Bass kernel for a single attention head with query-axis softmax.

Reference semantics (per batch b):
    k = x @ Wk; q = x @ Wq; v = x @ Wv                 # [T, H]
    wei = (q @ k^T) * E**-0.5                          # [T(query), T(key)]
    wei = where(tril, wei, -inf)                       # causal: keep s <= t
    p = softmax(wei, axis=0 over query t)              # NOTE: query axis!
    out = p @ v                                        # [T, H]

Because the softmax normalizes over the query axis t (per key column s),
out[t,h] = sum_s E[t,s] * v[s,h] / d[s] with E[t,s] = exp(wei[t,s])
(zero for s > t) and d[s] = sum_t E[t,s].  The kernel computes E^T tiles
([s on partitions, t free]) so d is a free-axis row sum (fused into the
exp instruction via accum_out), scales v rows by 1/d, and accumulates
out^T on PE.  out^T is stored as-is; the host un-transposes during the
gather (free), so no on-device layout fixup is needed.

The causal triangle mask on the diagonal block is applied ON the PE:
a 128-row matmul against the identity writes the additive -1e30 triangle
into PSUM (start of the accumulation group), and the diagonal S matmul
accumulates on top of it - no vector/gpsimd op, no cross-engine hop.

Projections: k and v are packed into one 128-partition stationary
([kT; vT] stacked, so kT and qT share partition base 0 as the S matmul
requires), q separate; two PSUM->SBUF casts per column block (kv, then
q) so the next phase's weight loads unblock early.  The v chunks are
re-transposed to natural [s, h] layout with regular 64-row matmuls
against the identity (cheaper than transpose-mode, and batched per
phase so the S-pair PSUM ring stays off vector's critical path).

Sharding: batch dim (8) across the 8 NeuronCores, weights replicated.
x is host-packed per column block ([NJ, 128, NE*CB] bf16).  A single
dma_start tops out at ~165 GB/s and per-stream rate falls as streams
are added while the aggregate rises, so each block moves as 2-3
concurrent streams spread over the idle queues (blocks 2/1 borrow the
scalar queue, which is free until the first exps), and blocks are paced
j=3..0 by tiny WAW fences on the gpsimd queue so early blocks get the
wire first.  Dummy matmuls bridge the PE from program start to the
first projection so the activity monitor never sees an idle window (14
of them: enough that the HAM clock gate opens ~+6us and stays open for
the whole matmul stream).  The out^T banks are stored as one transfer
each on the two HWDGE queues (sync/scalar): 1KB partition runs instead
of 512B halves, and gpsimd's software DGE - whose final packets drain
several us late - stays off the tail.  During the last column block the
projection PSUM pool is idle, so the 8 S-row pieces round-robin over
three 2-bank slots; each piece's bank then frees a full exp before its
reuse and the per-row exp->S serialization gap disappears.
"""

import os

import numpy as np
import ml_dtypes

import concourse.bass as bass
import concourse.tile as tile
from concourse import bacc, mybir
from concourse import bass_utils
B, T, E, H = 8, 2048, 1024, 64
P = 128                       # partitions
CB = 512                      # column block (t) width
NE = E // P                   # 8 contraction chunks for projections
NJ = T // CB                  # 4 column blocks
SCALE = float(E) ** -0.5      # note: embed**-0.5, not head_size**-0.5
MASK_NEG = -1.0e30
F32 = mybir.dt.float32
BF16 = mybir.dt.bfloat16
X = mybir.AxisListType.X
EXP = mybir.ActivationFunctionType.Exp
COPY = mybir.ActivationFunctionType.Copy

# packed weights tensor column offsets (all bf16)
WKV0 = 0                      # [P, NE*P]   chunk e: [Wk_e | Wv_e]
WQ0 = WKV0 + NE * P           # [P, NE*H]
MASK0 = WQ0 + NE * H          # [P, 4*P]    additive -1e30 triangles
ID0 = MASK0 + 4 * P           # [P, P]      identity
WALLW = ID0 + P


def _emit(tc, xb_d, wall_d, out_d):
    nc = tc.nc
    from contextlib import ExitStack

    with ExitStack() as ctx:
        singles = ctx.enter_context(tc.tile_pool(name="singles", bufs=1))
        epool = ctx.enter_context(tc.tile_pool(name="erow", bufs=9))
        dpool = ctx.enter_context(tc.tile_pool(name="dsmall", bufs=12))
        vpool = ctx.enter_context(tc.tile_pool(name="vrow", bufs=9))
        vspool = ctx.enter_context(tc.tile_pool(name="vstage", bufs=2))
        ps = ctx.enter_context(tc.tile_pool(name="ps", bufs=2, space="PSUM"))
        pproj_pool = ctx.enter_context(
            tc.tile_pool(name="pproj", bufs=1, space="PSUM")
        )
        pout = ctx.enter_context(tc.tile_pool(name="pout", bufs=1, space="PSUM"))

        # weights: k/q/v gate the first projections, so they load first;
        # masks+identity live in a separate tile fetched a bit later so
        # the x fence graph stays acyclic
        wall = singles.tile([P, MASK0], BF16, name="wall")
        wall2 = singles.tile([P, WALLW - MASK0], BF16, name="wall2")
        wkv = wall[:, WKV0 : WKV0 + NE * P]
        wq = wall[:, WQ0 : WQ0 + NE * H]
        masks = wall2[:, 0 : 4 * P]
        identb = wall2[:, 4 * P : 5 * P]

        # x blocks: two tiles per block (e-chunks 0-3 / 4-7) so the
        # projection's data deps are piece-granular, moved by 2-3
        # concurrent wire streams, with later blocks paced behind earlier
        # ones by tiny WAW fences on the gpsimd queue.
        HALF = NE * CB // 2
        xlo = {
            j: singles.tile([P, HALF], BF16, name=f"xlo{j}")
            for j in range(NJ)
        }
        xhi = {
            j: singles.tile([P, HALF], BF16, name=f"xhi{j}")
            for j in range(NJ)
        }
        junk = singles.tile([P, CB], BF16)
        nc.gpsimd.memset(junk[:], 1.0)
        # open with three streams (weights + both x3 halves): per-stream
        # rate drops with concurrency but aggregate rises, and everything
        # gates on max(weights, x3) anyway
        nc.sync.dma_start(out=wall[:], in_=wall_d[:, 0:MASK0])
        nc.scalar.dma_start(out=xlo[3][:], in_=xb_d[3][:, 0:HALF])
        nc.gpsimd.dma_start(out=xhi[3][:, 0 : HALF // 2], in_=xb_d[3][:, HALF : HALF + HALF // 2])
        nc.sync.dma_start(out=xhi[3][:, HALF // 2 :], in_=xb_d[3][:, HALF + HALF // 2 :])
        nc.scalar.dma_start(out=wall2[:], in_=wall_d[:, MASK0:])
        for j in (2, 1, 0):
            # cross-paired fences: block j's lo waits block j+1's hi and
            # vice versa, so block j starts only once block j+1 is done
            nc.gpsimd.tensor_copy(xlo[j][0:1, 0:1], xhi[j + 1][0:1, 0:1])
            nc.gpsimd.tensor_copy(xhi[j][0:1, 0:1], xlo[j + 1][0:1, 0:1])
            lo_end = HALF // 2
            nc.sync.dma_start(
                out=xlo[j][:, 0:lo_end], in_=xb_d[j][:, 0:lo_end]
            )
            if j == 0:
                # block 0's third stream: a second sync-queue issue
                # (same-queue transfers still run concurrently)
                nc.sync.dma_start(
                    out=xlo[j][:, HALF // 2 :],
                    in_=xb_d[j][:, HALF // 2 : HALF],
                )
            nc.gpsimd.dma_start(out=xhi[j][:], in_=xb_d[j][:, HALF:])
            if j in (2, 1):
                # blocks 2 and 1 are also urgent (projection drips): third
                # stream on the scalar queue, which stays idle until the
                # first exps land (block 0 would collide with them)
                nc.scalar.dma_start(
                    out=xlo[j][:, HALF // 2 :],
                    in_=xb_d[j][:, HALF // 2 : HALF],
                )


        # two dummy matmuls bridge the gap until the first projection so
        # the PE activity monitor sees a busy stream from the start (more
        # warmup wastes power-limited cycles; the projection chain itself
        # finishes the warm-up)
        pwarm = ps.tile([P, 2 * CB], F32, tag="ps", name="pwarm")
        for w in range(14):
            nc.tensor.matmul(
                pwarm[:, 0:CB],
                lhsT=junk[:, 0:P],
                rhs=junk[:],
                start=(w == 0),
                stop=(w == 13),
            )

        # per-block projected activations: [kT(0:64); vT(64:128)]; q lives
        # in one contiguous [64, T] tile (same partition base 0 as kT, as
        # the S matmul requires) so S matmuls can span two column blocks
        kqv = {
            j: singles.tile([P, CB], BF16, name=f"kqv{j}")
            for j in range(NJ)
        }
        # q partial sums live in BOTH partition halves (even chunks 0:64,
        # odd 64:128); the S stationary [kT; kT] contracts them so no zero
        # fill and no final add are needed
        q_sb = singles.tile([P, T], BF16, name="qsb")
        kdup = {
            j: singles.tile([P, CB], BF16, name=f"kdup{j}") for j in range(NJ)
        }

        # out^T accumulators packed 2 per bank: jj even rows 0:64, odd 64:128.
        # Accumulation groups on disjoint partition ranges of one bank are
        # fine on HW (per-element has_written); skip the sim's coarse check.
        pout_tiles = [
            pout.tile([P, CB], F32, tag=f"pt{a}", name=f"pt{a}") for a in range(2)
        ]
        outst = singles.tile([P, 2 * CB], BF16, name="outst")

        def pout_slice(jj, c0, c1):
            rb = H * (jj % 2)
            return pout_tiles[jj // 2][rb : rb + H, c0:c1]

        # deferred AV emission (lag behind S so PE never waits on the
        # d / v' chain): each entry = (r, j_of_row), d0, erow, vi
        pending_av = []

        def _av_one(rj, d0, erow, vi, jj):
            c = (jj - rj[1]) * CB
            lo = d0 if jj == rj[1] else 0
            nc.tensor.matmul(
                pout_slice(jj, lo, CB),
                lhsT=vi[:],
                rhs=erow[:, c + lo : c + CB],
                start=(jj == rj[1] and rj[0] == 0),
                stop=(rj[1] == 0 and rj[0] == 3),
                skip_group_check=True,
            )

        def close_bank(a):
            # stage out^T bank a PSUM->SBUF, split across vector and scalar
            # so the copy's latency halves, then store it
            half = outst[:, a * CB : (a + 1) * CB]
            nc.vector.tensor_copy(half[:, 0 : CB // 2], pout_tiles[a][:, 0 : CB // 2])
            nc.scalar.activation(
                out=half[:, CB // 2 : CB],
                in_=pout_tiles[a][:, CB // 2 : CB],
                func=COPY,
            )
            # one store per bank: 1KB partition runs (vs 512B halves) and
            # two fewer ~0.65us issue slots on the tail's critical path.
            # Both on HWDGE queues (sync/scalar) - gpsimd's software DGE
            # drains its final packets several us late.
            eng0 = nc.sync if a == 0 else nc.scalar
            eng0.dma_start(
                out=out_d[:, a * CB : (a + 1) * CB],
                in_=half[:],
            )

        def flush_av(final):
            if final:
                # all rows but the last have their v' ready well before the
                # final exp finishes, so emit them first; after vi of the
                # last row only its own four matmuls and the bank closes
                # remain on the critical path
                rows = list(pending_av)
                pending_av.clear()
                for rj, d0, erow, vi in rows[:-1]:
                    for jj in range(rj[1], NJ):
                        _av_one(rj, d0, erow, vi, jj)
                rj, d0, erow, vi = rows[-1]
                for jj in range(rj[1], 2):
                    _av_one(rj, d0, erow, vi, jj)
                close_bank(0)
                for jj in range(2, NJ):
                    _av_one(rj, d0, erow, vi, jj)
                close_bank(1)
                return
            rj, d0, erow, vi = pending_av.pop(0)
            for jj in range(rj[1], NJ):
                _av_one(rj, d0, erow, vi, jj)

        # projection matmul emission is spread through the PREVIOUS step's
        # rows so the PE instruction stream stays dense (HAM stays warm)
        def x_rhs(j, e):
            t_ = xlo[j] if e < NE // 2 else xhi[j]
            c = (e % (NE // 2)) * CB
            return t_[:, c : c + CB]

        def proj_thunks(j):
            pproj = pproj_pool.tile([P, 2 * CB], F32, tag="pp", name="pproj")

            def kv_mm(e):
                nc.tensor.matmul(
                    pproj[:, 0:CB],
                    lhsT=wkv[:, e * P : (e + 1) * P],
                    rhs=x_rhs(j, e),
                    start=(e == 0),
                    stop=(e == NE - 1),
                )

            def q_mm(e):
                # even chunks -> out partitions 0:64 (col group h0), odd ->
                # 64:128 (h64); adjacent even/odd matmuls run CONCURRENTLY
                # on the PE (same mechanism as the AV pairs), halving the
                # q-chain's array-half waste.  The S matmuls contract the
                # two partial sums against a [kT; kT] stationary.
                rb = H * (e % 2)
                nc.tensor.matmul(
                    pproj[rb : rb + H, CB : 2 * CB],
                    lhsT=wq[:, e * H : (e + 1) * H],
                    rhs=x_rhs(j, e),
                    start=(e < 2),
                    stop=(e >= NE - 2),
                    skip_group_check=True,
                )

            def cast_k():
                # kT casts + the [kT;kT] duplication DMA fire while the q
                # chain is still on the PE, hiding the shift latency
                nc.vector.tensor_copy(kqv[j][:], pproj[:, 0:CB])
                nc.vector.tensor_copy(kdup[j][0:H, :], pproj[0:H, 0:CB])
                nc.scalar.dma_start(out=kdup[j][H:P, :], in_=kdup[j][0:H, :])

            # lo e-chunks of both chains first so the hi-half DMA's
            # arrival is overlapped by useful work; for dripped blocks the
            # cast_k pseudo-thunk rides after the last kv matmul so the
            # kdup shift hides under the remaining q matmuls
            thunks = []
            for e in range(NE // 2):
                thunks.append(lambda e=e: kv_mm(e))
            for e in range(NE // 2):
                thunks.append(lambda e=e: q_mm(e))
            for e in range(NE // 2, NE):
                thunks.append(lambda e=e: kv_mm(e))
            if j != 3:
                thunks.append(cast_k)
            for e in range(NE // 2, NE):
                thunks.append(lambda e=e: q_mm(e))
            return pproj, thunks

        def proj_cast(j, pproj):
            if j == 3:
                nc.vector.tensor_copy(kqv[j][:], pproj[:, 0:CB])
                nc.vector.tensor_copy(kdup[j][0:H, :], pproj[0:H, 0:CB])
                nc.scalar.dma_start(out=kdup[j][H:P, :], in_=kdup[j][0:H, :])
            nc.vector.tensor_copy(
                q_sb[:, j * CB : (j + 1) * CB], pproj[:, CB : 2 * CB]
            )

        # --- main pipeline: column blocks in descending order --------------
        piece_ctr = [0]
        next_proj = []  # pending matmul thunks for step j-1's projections

        def drip_proj(k):
            for _ in range(min(k, len(next_proj))):
                next_proj.pop(0)()

        pproj, thunks = proj_thunks(3)
        for t in thunks:
            t()
        proj_cast(3, pproj)

        for j in reversed(range(NJ)):
            if j > 0:
                pproj_next, next_proj = proj_thunks(j - 1)

            # batch-transpose the step's four v chunks into one PSUM tile
            # and stage them in SBUF unscaled: the per-row v' scale then
            # becomes a cheap SBUF-only multiply, and the S-pair PSUM ring
            # no longer threads through vector's per-row work.  The
            # transpose is a REGULAR matmul against the identity (cost 64
            # rows each, and no transpose-mode LDWEIGHTS, which would be
            # incompatible with walrus's ldw-opt): out[s,h] =
            # sum_h' vT[h',s] I[h',h]
            vps = ps.tile([P, 2 * CB], F32, tag="ps", name="vps")
            vstage = vspool.tile([P, 4 * H], BF16, name="vstage")

            # rows i = 4j .. 4j+3 of E^T are now computable in full
            for r in range(4):
                i = 4 * j + r
                d0 = r * P  # local offset of this s-chunk within block j
                nblk = NJ - j
                npair = (nblk + 1) // 2
                erow = epool.tile([P, T], BF16)
                dparts = dpool.tile([P, 2], F32, tag="dparts")
                kT_sl = kdup[j][:, d0 : d0 + P]

                psts = []
                for pair in range(npair):
                    jj0 = j + 2 * pair
                    w = CB * min(2, NJ - jj0)  # 512 or 1024
                    if j == 0:
                        # the projection pool is idle during the last step:
                        # round-robin the 8 pieces over THREE 2-bank slots
                        # (ps.A, ps.B, pproj) so each piece's bank frees a
                        # full exp earlier than its reuse - removes the
                        # exp->S serialization gap per j=0 row
                        k = piece_ctr[0]
                        piece_ctr[0] += 1
                        if k % 3 == 2:
                            pst = pproj_pool.tile(
                                [P, 2 * CB], F32, tag="pp", name="pst0"
                            )
                        else:
                            pst = ps.tile([P, 2 * CB], F32, tag="ps")
                    else:
                        pst = ps.tile([P, 2 * CB], F32, tag="ps")
                    psts.append((pst, jj0, w))

                # additive -1e30 triangle into PSUM via the PE (identity
                # stationary), then all S matmuls back-to-back with the
                # same kT stationary (single weight load with ldw-opt)
                pst0 = psts[0][0]
                nc.tensor.matmul(
                    pst0[:, d0 : d0 + P],
                    lhsT=identb,
                    rhs=masks[:, r * P : (r + 1) * P],
                    start=True,
                    stop=False,
                )
                for pair, (pst, jj0, w) in enumerate(psts):
                    if j == 0 and pair == 1 and pending_av:
                        # the borrowed pair-1 tile serializes on the prior
                        # row's exp; keep ready AV work ahead of it in the
                        # queue so the PE never drains
                        flush_av(False)
                    t0 = jj0 * CB  # global t of this pair's first column
                    lo0 = d0 if pair == 0 else 0
                    if pair == 0:
                        # diagonal block: accumulate onto the mask first
                        nc.tensor.matmul(
                            pst[:, d0 : d0 + P],
                            lhsT=kT_sl,
                            rhs=q_sb[:, t0 + d0 : t0 + d0 + P],
                            start=False,
                            stop=True,
                        )
                        lo0 = d0 + P
                    # the rest in <=512-column pieces (the ISA caps a
                    # matmul's output at one PSUM bank)
                    c = lo0
                    while c < w:
                        ce = min(c + CB - c % CB, w)
                        nc.tensor.matmul(
                            pst[:, c:ce],
                            lhsT=kT_sl,
                            rhs=q_sb[:, t0 + c : t0 + ce],
                            start=True,
                            stop=True,
                        )
                        c = ce
                if r == 0:
                    # the v transposes are not needed until after row 0's
                    # exp, so they run BEHIND row 0's S matmuls: the phase
                    # boundary then feeds the scalar exp stream (the tail's
                    # clock) as early as possible.  Full 128-row stationary
                    # (FWL): rows 0:64 of the identity slice are zero, so
                    # the kT half contributes nothing.
                    for rr in range(4):
                        nc.tensor.matmul(
                            vps[:, rr * H : (rr + 1) * H],
                            lhsT=kqv[j][:, rr * P : (rr + 1) * P],
                            rhs=identb[:, H:P],
                            start=True,
                            stop=True,
                        )
                    nc.vector.tensor_copy(vstage[:], vps[:, 0 : 4 * H])
                drip_proj(3)

                # exp (+ d partial sums fused via accum_out), per pair.
                # For the very last row the pairs are swapped so the final
                # (tail-gating) activation is the short one
                pairs_e = list(enumerate(psts))
                if j == 0 and r == 3:
                    pairs_e.reverse()
                for pair, (pst, jj0, w) in pairs_e:
                    lo = d0 if pair == 0 else 0
                    c = 2 * CB * pair
                    nc.scalar.activation(
                        out=erow[:, c + lo : c + w],
                        in_=pst[:, lo:w],
                        func=EXP,
                        scale=SCALE,
                        accum_out=dparts[:, pair : pair + 1],
                    )

                # d = sum over the row; 1/d feeds the v' scale
                dinv = dpool.tile([P, 1], F32, tag="dinv")
                if npair > 1:
                    # plain add beats tensor_reduce (the slowest DVE op)
                    # on this 2-element sum, and it sits on the tail-
                    # critical d chain
                    dsum = dpool.tile([P, 1], F32, tag="dsum")
                    nc.vector.tensor_add(
                        dsum[:], dparts[:, 0:1], dparts[:, 1:2]
                    )
                    nc.vector.reciprocal(dinv[:], dsum[:])
                else:
                    nc.vector.reciprocal(dinv[:], dparts[:, 0:1])

                vi = vpool.tile([P, H], BF16, tag="vi", name="vi")
                nc.vector.tensor_scalar_mul(
                    vi[:], vstage[:, r * H : (r + 1) * H], dinv[:]
                )

                lag = 5 if j == 1 else (2 if j == 0 else 2)
                if len(pending_av) >= lag:
                    flush_av(False)  # AV matmuls lag behind S for overlap
                drip_proj(2)
                pending_av.append(((r, j), d0, erow, vi))

            # drain remaining next-step projection matmuls, then its cast
            drip_proj(len(next_proj))
            if j > 0:
                proj_cast(j - 1, pproj_next)

        flush_av(True)


def _enable_ldw_opt():
    """Flip walrus's --enable-ldw-opt to true for our compile: consecutive
    matmuls reusing the same stationary operand then skip the reload."""
    import concourse.bass_utils as bu

    if getattr(bu, "_ldw_opt_patched", False):
        return
    orig = bu.run_command

    def run_command_ldw(cmd, *a, **kw):
        if isinstance(cmd, list):
            cmd = [
                "--enable-ldw-opt=true" if c == "--enable-ldw-opt=false" else c
                for c in cmd
            ]
        return orig(cmd, *a, **kw)

    bu.run_command = run_command_ldw
    bu._ldw_opt_patched = True


def _build_program():
    # walrus rejects --enable-ldw-opt=true for transpose-mode LDWEIGHTS;
    # the kernel avoids transpose matmuls entirely so the opt is safe and
    # skips the stationary reload for back-to-back same-lhsT matmuls
    if os.environ.get("BASS_LDW_OPT", "0") == "1":
        _enable_ldw_opt()
    nc = bacc.Bacc("TRN2", target_bir_lowering=False, debug=False, num_devices=B)
    xb_d = nc.dram_tensor("xb", [NJ, P, NE * CB], BF16, kind="ExternalInput").ap()
    wall_d = nc.dram_tensor("wall", [P, WALLW], BF16, kind="ExternalInput").ap()
    out_d = nc.dram_tensor("out", [P, 2 * CB], BF16, kind="ExternalOutput").ap()
    with tile.TileContext(nc) as tc:
        _emit(tc, xb_d, wall_d, out_d)
    nc.compile()
    return nc


def _host_masks():
    """[128, 4*128] additive triangles: row r masks t < s within the
    diagonal 128-block (t-local f, partition p: keep f >= p)."""
    m = np.full((P, 4 * P), MASK_NEG, dtype=np.float32)
    p = np.arange(P)[:, None]
    f = np.arange(P)[None, :]
    for r in range(4):
        m[:, r * P : (r + 1) * P][f >= p] = 0.0
    return m


def _host_inputs(x, Wk, Wq, Wv):
    bf = ml_dtypes.bfloat16
    x = np.asarray(x, dtype=np.float32)
    # [B, E, T] -> block-major [B, NJ, P, NE*CB] so each block is one
    # contiguous DMA with 4KB/partition runs
    xT = np.transpose(x, (0, 2, 1)).reshape(B, NE, P, NJ, CB)
    xb = np.ascontiguousarray(xT.transpose(0, 3, 2, 1, 4)).reshape(
        B, NJ, P, NE * CB
    ).astype(bf)

    def chunks(w):  # [E, h] -> [NE, P, h]
        return np.asarray(w, np.float32).reshape(NE, P, -1)

    # wkv chunk e = [Wk_e | Wv_e] -> [P, NE*128]
    kv = np.concatenate([chunks(Wk), chunks(Wv)], axis=2)
    wkv = kv.transpose(1, 0, 2).reshape(P, NE * P)
    wqp = chunks(Wq).transpose(1, 0, 2).reshape(P, NE * H)
    wall = np.concatenate(
        [wkv, wqp, _host_masks(), np.eye(P, dtype=np.float32)], axis=1
    ).astype(bf)
    assert wall.shape == (P, WALLW)
    return [{"xb": xb[b], "wall": wall} for b in range(B)]


def _unpack_out(outT):
    """[128, 1024] out^T banks (bf16) -> [T, H] f32 natural layout."""
    outT = np.asarray(outT, dtype=np.float32)
    o = np.empty((T, H), dtype=np.float32)
    for a in range(2):
        for h2 in range(2):
            jj = 2 * a + h2
            o[jj * CB : (jj + 1) * CB, :] = outT[
                H * h2 : H * (h2 + 1), a * CB : (a + 1) * CB
            ].T
    return o


def _ensure_axon_ntff_hook():
    """The agent image's antenv lacks axon_hooks; synthesize it so
    run_bass_kernel_spmd's trace path can find the NTFF profile hook."""
    import sys
    import types

    if "antenv.axon_hooks" in sys.modules:
        return
    try:
        import antenv

        mod = types.ModuleType("antenv.axon_hooks")
        mod._hook = None

        def set_axon_ntff_profile_hook(h):
            mod._hook = h

        def get_axon_ntff_profile_hook():
            return mod._hook

        mod.set_axon_ntff_profile_hook = set_axon_ntff_profile_hook
        mod.get_axon_ntff_profile_hook = get_axon_ntff_profile_hook
        sys.modules["antenv.axon_hooks"] = mod
        antenv.axon_hooks = mod

        from trn_agent_boot.trn_boot import _ntff_profile_via_ctypes

        hook = _ntff_profile_via_ctypes("/opt/axon/libaxon_pjrt.so")
        if hook is not None:
            mod._hook = hook
    except Exception as e:  # degrade to untraced run
        print(f"NTFF hook setup failed ({e}); tracing will be skipped")


def kernel(x, Wk, Wq, Wv, _trace=False, _trace_kwargs=None):
    if _trace:
        _ensure_axon_ntff_hook()
    in_maps = _host_inputs(x, Wk, Wq, Wv)
    nc = _build_program()
    res = bass_utils.run_bass_kernel_spmd(
        nc, in_maps, list(range(B)), trace=_trace, **(_trace_kwargs or {})
    )
    out = np.stack(
        [_unpack_out(res.results[b]["out"]) for b in range(B)], axis=0
    )
    if _trace:
        kernel.last_results = res
    return out.astype(np.float32)



# revision 34
# speedup vs baseline: 1.1608x; 1.0343x over previous
"""Trainium2 Bass kernel for a single attention head with query-axis softmax.

Reference semantics (per batch b):
    k = x @ Wk; q = x @ Wq; v = x @ Wv                 # [T, H]
    wei = (q @ k^T) * E**-0.5                          # [T(query), T(key)]
    wei = where(tril, wei, -inf)                       # causal: keep s <= t
    p = softmax(wei, axis=0 over query t)              # NOTE: query axis!
    out = p @ v                                        # [T, H]

Because the softmax normalizes over the query axis t (per key column s),
out[t,h] = sum_s E[t,s] * v[s,h] / d[s] with E[t,s] = exp(wei[t,s])
(zero for s > t) and d[s] = sum_t E[t,s].  The kernel computes E^T tiles
([s on partitions, t free]) so d is a free-axis row sum (fused into the
exp instruction via accum_out), scales v rows by 1/d, and accumulates
out^T on PE.  out^T is stored as-is; the host un-transposes during the
gather (free), so no on-device layout fixup is needed.

The causal triangle mask on the diagonal block is applied ON the PE:
a 128-row matmul against the identity writes the additive -1e30 triangle
into PSUM (start of the accumulation group), and the diagonal S matmul
accumulates on top of it - no vector/gpsimd op, no cross-engine hop.

Projections: k and v are packed into one 128-partition stationary
([kT; vT] stacked, so kT and qT share partition base 0 as the S matmul
requires), q separate; two PSUM->SBUF casts per column block (kv, then
q) so the next phase's weight loads unblock early.  The v chunks are
re-transposed to natural [s, h] layout with regular 64-row matmuls
against the identity (cheaper than transpose-mode, and batched per
phase so the S-pair PSUM ring stays off vector's critical path).

Sharding: batch dim (8) across the 8 NeuronCores, weights replicated.
x is host-packed per column block ([NJ, 128, NE*CB] bf16).  A single
dma_start tops out at ~165 GB/s and per-stream rate falls as streams
are added while the aggregate rises, so each block moves as 2-3
concurrent streams spread over the idle queues (blocks 2/1 borrow the
scalar queue, which is free until the first exps), and blocks are paced
j=3..0 by tiny WAW fences on the gpsimd queue so early blocks get the
wire first.  Dummy matmuls bridge the PE from program start to the
first projection so the activity monitor never sees an idle window (14
of them: enough that the HAM clock gate opens ~+6us and stays open for
the whole matmul stream).  The out^T banks are stored as one transfer
each on the two HWDGE queues (sync/scalar): 1KB partition runs instead
of 512B halves, and gpsimd's software DGE - whose final packets drain
several us late - stays off the tail.  During the last column block the
projection PSUM pool is idle, so the 8 S-row pieces round-robin over
three 2-bank slots; each piece's bank then frees a full exp before its
reuse and the per-row exp->S serialization gap disappears.
"""

import os

import numpy as np
import ml_dtypes

import concourse.bass as bass
import concourse.tile as tile
from concourse import bacc, mybir
from concourse import bass_utils
B, T, E, H = 8, 2048, 1024, 64
P = 128                       # partitions
CB = 512                      # column block (t) width
NE = E // P                   # 8 contraction chunks for projections
NJ = T // CB                  # 4 column blocks
SCALE = float(E) ** -0.5      # note: embed**-0.5, not head_size**-0.5
MASK_NEG = -1.0e30
F32 = mybir.dt.float32
BF16 = mybir.dt.bfloat16
X = mybir.AxisListType.X
EXP = mybir.ActivationFunctionType.Exp
COPY = mybir.ActivationFunctionType.Copy

# packed weights tensor column offsets (all bf16)
WKV0 = 0                      # [P, NE*P]   chunk e: [Wk_e | Wv_e]
WQ0 = WKV0 + NE * P           # [P, NE*H]
MASK0 = WQ0 + NE * H          # [P, 4*P]    additive -1e30 triangles
ID0 = MASK0 + 4 * P           # [P, P]      identity
WALLW = ID0 + P


def _emit(tc, xb_d, wall_d, out_d):
    nc = tc.nc
    from contextlib import ExitStack

    with ExitStack() as ctx:
        singles = ctx.enter_context(tc.tile_pool(name="singles", bufs=1))
        epool = ctx.enter_context(tc.tile_pool(name="erow", bufs=9))
        dpool = ctx.enter_context(tc.tile_pool(name="dsmall", bufs=12))
        vpool = ctx.enter_context(tc.tile_pool(name="vrow", bufs=9))
        vspool = ctx.enter_context(tc.tile_pool(name="vstage", bufs=2))
        ps = ctx.enter_context(tc.tile_pool(name="ps", bufs=2, space="PSUM"))
        pproj_pool = ctx.enter_context(
            tc.tile_pool(name="pproj", bufs=1, space="PSUM")
        )
        pout = ctx.enter_context(tc.tile_pool(name="pout", bufs=1, space="PSUM"))

        # weights: k/q/v gate the first projections, so they load first;
        # masks+identity live in a separate tile fetched a bit later so
        # the x fence graph stays acyclic
        wall = singles.tile([P, MASK0], BF16, name="wall")
        wall2 = singles.tile([P, WALLW - MASK0], BF16, name="wall2")
        wkv = wall[:, WKV0 : WKV0 + NE * P]
        wq = wall[:, WQ0 : WQ0 + NE * H]
        masks = wall2[:, 0 : 4 * P]
        identb = wall2[:, 4 * P : 5 * P]

        # x blocks: two tiles per block (e-chunks 0-3 / 4-7) so the
        # projection's data deps are piece-granular, moved by 2-3
        # concurrent wire streams, with later blocks paced behind earlier
        # ones by tiny WAW fences on the gpsimd queue.
        HALF = NE * CB // 2
        xlo = {
            j: singles.tile([P, HALF], BF16, name=f"xlo{j}")
            for j in range(NJ)
        }
        xhi = {
            j: singles.tile([P, HALF], BF16, name=f"xhi{j}")
            for j in range(NJ)
        }
        junk = singles.tile([P, CB], BF16)
        nc.gpsimd.memset(junk[:], 1.0)
        # open with three streams (weights + both x3 halves): per-stream
        # rate drops with concurrency but aggregate rises, and everything
        # gates on max(weights, x3) anyway
        nc.sync.dma_start(out=wall[:], in_=wall_d[:, 0:MASK0])
        nc.scalar.dma_start(out=xlo[3][:], in_=xb_d[3][:, 0:HALF])
        nc.gpsimd.dma_start(out=xhi[3][:, 0 : HALF // 2], in_=xb_d[3][:, HALF : HALF + HALF // 2])
        nc.sync.dma_start(out=xhi[3][:, HALF // 2 :], in_=xb_d[3][:, HALF + HALF // 2 :])
        nc.scalar.dma_start(out=wall2[:], in_=wall_d[:, MASK0:])
        for j in (2, 1, 0):
            # cross-paired fences: block j's lo waits block j+1's hi and
            # vice versa, so block j starts only once block j+1 is done
            nc.gpsimd.tensor_copy(xlo[j][0:1, 0:1], xhi[j + 1][0:1, 0:1])
            nc.gpsimd.tensor_copy(xhi[j][0:1, 0:1], xlo[j + 1][0:1, 0:1])
            lo_end = HALF // 2
            nc.sync.dma_start(
                out=xlo[j][:, 0:lo_end], in_=xb_d[j][:, 0:lo_end]
            )
            if j == 0:
                # block 0's third stream: a second sync-queue issue
                # (same-queue transfers still run concurrently)
                nc.sync.dma_start(
                    out=xlo[j][:, HALF // 2 :],
                    in_=xb_d[j][:, HALF // 2 : HALF],
                )
            nc.gpsimd.dma_start(out=xhi[j][:], in_=xb_d[j][:, HALF:])
            if j in (2, 1):
                # blocks 2 and 1 are also urgent (projection drips): third
                # stream on the scalar queue, which stays idle until the
                # first exps land (block 0 would collide with them)
                nc.scalar.dma_start(
                    out=xlo[j][:, HALF // 2 :],
                    in_=xb_d[j][:, HALF // 2 : HALF],
                )


        # two dummy matmuls bridge the gap until the first projection so
        # the PE activity monitor sees a busy stream from the start (more
        # warmup wastes power-limited cycles; the projection chain itself
        # finishes the warm-up)
        pwarm = ps.tile([P, 2 * CB], F32, tag="ps", name="pwarm")
        for w in range(14):
            nc.tensor.matmul(
                pwarm[:, 0:CB],
                lhsT=junk[:, 0:P],
                rhs=junk[:],
                start=(w == 0),
                stop=(w == 13),
            )

        # per-block projected activations: [kT(0:64); vT(64:128)]; q lives
        # in one contiguous [64, T] tile (same partition base 0 as kT, as
        # the S matmul requires) so S matmuls can span two column blocks
        kqv = {
            j: singles.tile([P, CB], BF16, name=f"kqv{j}")
            for j in range(NJ)
        }
        # q lives in rows 0:64; rows 64:128 are zeroed once so the S
        # matmuls can take the full 128-row [kT; vT] slice as stationary
        # (vT multiplies zeros): a 128-row stationary enables the PE's
        # automatic Fast Weight Load (2x faster LDWEIGHTS)
        q_sb = singles.tile([P, T], BF16, name="qsb")
        nc.vector.memset(q_sb[H:P, :], 0.0)

        # out^T accumulators packed 2 per bank: jj even rows 0:64, odd 64:128.
        # Accumulation groups on disjoint partition ranges of one bank are
        # fine on HW (per-element has_written); skip the sim's coarse check.
        pout_tiles = [
            pout.tile([P, CB], F32, tag=f"pt{a}", name=f"pt{a}") for a in range(2)
        ]
        outst = singles.tile([P, 2 * CB], BF16, name="outst")

        def pout_slice(jj, c0, c1):
            rb = H * (jj % 2)
            return pout_tiles[jj // 2][rb : rb + H, c0:c1]

        # deferred AV emission (lag behind S so PE never waits on the
        # d / v' chain): each entry = (r, j_of_row), d0, erow, vi
        pending_av = []

        def _av_one(rj, d0, erow, vi, jj):
            c = (jj - rj[1]) * CB
            lo = d0 if jj == rj[1] else 0
            nc.tensor.matmul(
                pout_slice(jj, lo, CB),
                lhsT=vi[:],
                rhs=erow[:, c + lo : c + CB],
                start=(jj == rj[1] and rj[0] == 0),
                stop=(rj[1] == 0 and rj[0] == 3),
                skip_group_check=True,
            )

        def close_bank(a):
            # stage out^T bank a PSUM->SBUF, split across vector and scalar
            # so the copy's latency halves, then store it
            half = outst[:, a * CB : (a + 1) * CB]
            nc.vector.tensor_copy(half[:, 0 : CB // 2], pout_tiles[a][:, 0 : CB // 2])
            nc.scalar.activation(
                out=half[:, CB // 2 : CB],
                in_=pout_tiles[a][:, CB // 2 : CB],
                func=COPY,
            )
            # one store per bank: 1KB partition runs (vs 512B halves) and
            # two fewer ~0.65us issue slots on the tail's critical path.
            # Both on HWDGE queues (sync/scalar) - gpsimd's software DGE
            # drains its final packets several us late.
            eng0 = nc.sync if a == 0 else nc.scalar
            eng0.dma_start(
                out=out_d[:, a * CB : (a + 1) * CB],
                in_=half[:],
            )

        def flush_av(final):
            if final:
                # all rows but the last have their v' ready well before the
                # final exp finishes, so emit them first; after vi of the
                # last row only its own four matmuls and the bank closes
                # remain on the critical path
                rows = list(pending_av)
                pending_av.clear()
                for rj, d0, erow, vi in rows[:-1]:
                    for jj in range(rj[1], NJ):
                        _av_one(rj, d0, erow, vi, jj)
                rj, d0, erow, vi = rows[-1]
                for jj in range(rj[1], 2):
                    _av_one(rj, d0, erow, vi, jj)
                close_bank(0)
                for jj in range(2, NJ):
                    _av_one(rj, d0, erow, vi, jj)
                close_bank(1)
                return
            rj, d0, erow, vi = pending_av.pop(0)
            for jj in range(rj[1], NJ):
                _av_one(rj, d0, erow, vi, jj)

        # projection matmul emission is spread through the PREVIOUS step's
        # rows so the PE instruction stream stays dense (HAM stays warm)
        def x_rhs(j, e):
            t_ = xlo[j] if e < NE // 2 else xhi[j]
            c = (e % (NE // 2)) * CB
            return t_[:, c : c + CB]

        def proj_thunks(j):
            pproj = pproj_pool.tile([P, 2 * CB], F32, tag="pp", name="pproj")

            def kv_mm(e):
                nc.tensor.matmul(
                    pproj[:, 0:CB],
                    lhsT=wkv[:, e * P : (e + 1) * P],
                    rhs=x_rhs(j, e),
                    start=(e == 0),
                    stop=(e == NE - 1),
                )

            def q_mm(e):
                nc.tensor.matmul(
                    pproj[0:H, CB : 2 * CB],
                    lhsT=wq[:, e * H : (e + 1) * H],
                    rhs=x_rhs(j, e),
                    start=(e == 0),
                    stop=(e == NE - 1),
                )

            # lo e-chunks of both chains first, so the hi-half DMA's
            # arrival is overlapped by useful work instead of stalling
            # the kv chain midway
            thunks = []
            for e in range(NE // 2):
                thunks.append(lambda e=e: kv_mm(e))
            for e in range(NE // 2):
                thunks.append(lambda e=e: q_mm(e))
            for e in range(NE // 2, NE):
                thunks.append(lambda e=e: kv_mm(e))
            for e in range(NE // 2, NE):
                thunks.append(lambda e=e: q_mm(e))
            return pproj, thunks

        def proj_cast(j, pproj):
            # split casts (both on vector): the kv half unblocks the next
            # phase's kT weight loads and v transposes ~0.6us before the
            # q half finishes
            nc.vector.tensor_copy(kqv[j][:], pproj[:, 0:CB])
            nc.vector.tensor_copy(
                q_sb[0:H, j * CB : (j + 1) * CB], pproj[0:H, CB : 2 * CB]
            )

        # --- main pipeline: column blocks in descending order --------------
        piece_ctr = [0]
        next_proj = []  # pending matmul thunks for step j-1's projections

        def drip_proj(k):
            for _ in range(min(k, len(next_proj))):
                next_proj.pop(0)()

        pproj, thunks = proj_thunks(3)
        for t in thunks:
            t()
        proj_cast(3, pproj)

        for j in reversed(range(NJ)):
            if j > 0:
                pproj_next, next_proj = proj_thunks(j - 1)

            # batch-transpose the step's four v chunks into one PSUM tile
            # and stage them in SBUF unscaled: the per-row v' scale then
            # becomes a cheap SBUF-only multiply, and the S-pair PSUM ring
            # no longer threads through vector's per-row work.  The
            # transpose is a REGULAR matmul against the identity (cost 64
            # rows each, and no transpose-mode LDWEIGHTS, which would be
            # incompatible with walrus's ldw-opt): out[s,h] =
            # sum_h' vT[h',s] I[h',h]
            vps = ps.tile([P, 2 * CB], F32, tag="ps", name="vps")
            vstage = vspool.tile([P, 4 * H], BF16, name="vstage")

            # rows i = 4j .. 4j+3 of E^T are now computable in full
            for r in range(4):
                i = 4 * j + r
                d0 = r * P  # local offset of this s-chunk within block j
                nblk = NJ - j
                npair = (nblk + 1) // 2
                erow = epool.tile([P, T], BF16)
                dparts = dpool.tile([P, 2], F32, tag="dparts")
                kT_sl = kqv[j][:, d0 : d0 + P]

                psts = []
                for pair in range(npair):
                    jj0 = j + 2 * pair
                    w = CB * min(2, NJ - jj0)  # 512 or 1024
                    if j == 0:
                        # the projection pool is idle during the last step:
                        # round-robin the 8 pieces over THREE 2-bank slots
                        # (ps.A, ps.B, pproj) so each piece's bank frees a
                        # full exp earlier than its reuse - removes the
                        # exp->S serialization gap per j=0 row
                        k = piece_ctr[0]
                        piece_ctr[0] += 1
                        if k % 3 == 2:
                            pst = pproj_pool.tile(
                                [P, 2 * CB], F32, tag="pp", name="pst0"
                            )
                        else:
                            pst = ps.tile([P, 2 * CB], F32, tag="ps")
                    else:
                        pst = ps.tile([P, 2 * CB], F32, tag="ps")
                    psts.append((pst, jj0, w))

                # additive -1e30 triangle into PSUM via the PE (identity
                # stationary), then all S matmuls back-to-back with the
                # same kT stationary (single weight load with ldw-opt)
                pst0 = psts[0][0]
                nc.tensor.matmul(
                    pst0[:, d0 : d0 + P],
                    lhsT=identb,
                    rhs=masks[:, r * P : (r + 1) * P],
                    start=True,
                    stop=False,
                )
                for pair, (pst, jj0, w) in enumerate(psts):
                    if j == 0 and pair == 1 and pending_av:
                        # the borrowed pair-1 tile serializes on the prior
                        # row's exp; keep ready AV work ahead of it in the
                        # queue so the PE never drains
                        flush_av(False)
                    t0 = jj0 * CB  # global t of this pair's first column
                    lo0 = d0 if pair == 0 else 0
                    if pair == 0:
                        # diagonal block: accumulate onto the mask first
                        nc.tensor.matmul(
                            pst[:, d0 : d0 + P],
                            lhsT=kT_sl,
                            rhs=q_sb[:, t0 + d0 : t0 + d0 + P],
                            start=False,
                            stop=True,
                        )
                        lo0 = d0 + P
                    # the rest in <=512-column pieces (the ISA caps a
                    # matmul's output at one PSUM bank)
                    c = lo0
                    while c < w:
                        ce = min(c + CB - c % CB, w)
                        nc.tensor.matmul(
                            pst[:, c:ce],
                            lhsT=kT_sl,
                            rhs=q_sb[:, t0 + c : t0 + ce],
                            start=True,
                            stop=True,
                        )
                        c = ce
                if r == 0:
                    # the v transposes are not needed until after row 0's
                    # exp, so they run BEHIND row 0's S matmuls: the phase
                    # boundary then feeds the scalar exp stream (the tail's
                    # clock) as early as possible.  Full 128-row stationary
                    # (FWL): rows 0:64 of the identity slice are zero, so
                    # the kT half contributes nothing.
                    for rr in range(4):
                        nc.tensor.matmul(
                            vps[:, rr * H : (rr + 1) * H],
                            lhsT=kqv[j][:, rr * P : (rr + 1) * P],
                            rhs=identb[:, H:P],
                            start=True,
                            stop=True,
                        )
                    nc.vector.tensor_copy(vstage[:], vps[:, 0 : 4 * H])
                drip_proj(3)

                # exp (+ d partial sums fused via accum_out), per pair.
                # For the very last row the pairs are swapped so the final
                # (tail-gating) activation is the short one
                pairs_e = list(enumerate(psts))
                if j == 0 and r == 3:
                    pairs_e.reverse()
                for pair, (pst, jj0, w) in pairs_e:
                    lo = d0 if pair == 0 else 0
                    c = 2 * CB * pair
                    nc.scalar.activation(
                        out=erow[:, c + lo : c + w],
                        in_=pst[:, lo:w],
                        func=EXP,
                        scale=SCALE,
                        accum_out=dparts[:, pair : pair + 1],
                    )

                # d = sum over the row; 1/d feeds the v' scale
                dinv = dpool.tile([P, 1], F32, tag="dinv")
                if npair > 1:
                    # plain add beats tensor_reduce (the slowest DVE op)
                    # on this 2-element sum, and it sits on the tail-
                    # critical d chain
                    dsum = dpool.tile([P, 1], F32, tag="dsum")
                    nc.vector.tensor_add(
                        dsum[:], dparts[:, 0:1], dparts[:, 1:2]
                    )
                    nc.vector.reciprocal(dinv[:], dsum[:])
                else:
                    nc.vector.reciprocal(dinv[:], dparts[:, 0:1])

                vi = vpool.tile([P, H], BF16, tag="vi", name="vi")
                nc.vector.tensor_scalar_mul(
                    vi[:], vstage[:, r * H : (r + 1) * H], dinv[:]
                )

                lag = 5 if j == 1 else (2 if j == 0 else 2)
                if len(pending_av) >= lag:
                    flush_av(False)  # AV matmuls lag behind S for overlap
                drip_proj(2)
                pending_av.append(((r, j), d0, erow, vi))

            # drain remaining next-step projection matmuls, then its cast
            drip_proj(len(next_proj))
            if j > 0:
                proj_cast(j - 1, pproj_next)

        flush_av(True)


def _enable_ldw_opt():
    """Flip walrus's --enable-ldw-opt to true for our compile: consecutive
    matmuls reusing the same stationary operand then skip the reload."""
    import concourse.bass_utils as bu

    if getattr(bu, "_ldw_opt_patched", False):
        return
    orig = bu.run_command

    def run_command_ldw(cmd, *a, **kw):
        if isinstance(cmd, list):
            cmd = [
                "--enable-ldw-opt=true" if c == "--enable-ldw-opt=false" else c
                for c in cmd
            ]
        return orig(cmd, *a, **kw)

    bu.run_command = run_command_ldw
    bu._ldw_opt_patched = True


def _build_program():
    # walrus rejects --enable-ldw-opt=true for transpose-mode LDWEIGHTS;
    # the kernel avoids transpose matmuls entirely so the opt is safe and
    # skips the stationary reload for back-to-back same-lhsT matmuls
    if os.environ.get("BASS_LDW_OPT", "0") == "1":
        _enable_ldw_opt()
    nc = bacc.Bacc("TRN2", target_bir_lowering=False, debug=False, num_devices=B)
    xb_d = nc.dram_tensor("xb", [NJ, P, NE * CB], BF16, kind="ExternalInput").ap()
    wall_d = nc.dram_tensor("wall", [P, WALLW], BF16, kind="ExternalInput").ap()
    out_d = nc.dram_tensor("out", [P, 2 * CB], BF16, kind="ExternalOutput").ap()
    with tile.TileContext(nc) as tc:
        _emit(tc, xb_d, wall_d, out_d)
    nc.compile()
    return nc


def _host_masks():
    """[128, 4*128] additive triangles: row r masks t < s within the
    diagonal 128-block (t-local f, partition p: keep f >= p)."""
    m = np.full((P, 4 * P), MASK_NEG, dtype=np.float32)
    p = np.arange(P)[:, None]
    f = np.arange(P)[None, :]
    for r in range(4):
        m[:, r * P : (r + 1) * P][f >= p] = 0.0
    return m


def _host_inputs(x, Wk, Wq, Wv):
    bf = ml_dtypes.bfloat16
    x = np.asarray(x, dtype=np.float32)
    # [B, E, T] -> block-major [B, NJ, P, NE*CB] so each block is one
    # contiguous DMA with 4KB/partition runs
    xT = np.transpose(x, (0, 2, 1)).reshape(B, NE, P, NJ, CB)
    xb = np.ascontiguousarray(xT.transpose(0, 3, 2, 1, 4)).reshape(
        B, NJ, P, NE * CB
    ).astype(bf)

    def chunks(w):  # [E, h] -> [NE, P, h]
        return np.asarray(w, np.float32).reshape(NE, P, -1)

    # wkv chunk e = [Wk_e | Wv_e] -> [P, NE*128]
    kv = np.concatenate([chunks(Wk), chunks(Wv)], axis=2)
    wkv = kv.transpose(1, 0, 2).reshape(P, NE * P)
    wqp = chunks(Wq).transpose(1, 0, 2).reshape(P, NE * H)
    wall = np.concatenate(
        [wkv, wqp, _host_masks(), np.eye(P, dtype=np.float32)], axis=1
    ).astype(bf)
    assert wall.shape == (P, WALLW)
    return [{"xb": xb[b], "wall": wall} for b in range(B)]


def _unpack_out(outT):
    """[128, 1024] out^T banks (bf16) -> [T, H] f32 natural layout."""
    outT = np.asarray(outT, dtype=np.float32)
    o = np.empty((T, H), dtype=np.float32)
    for a in range(2):
        for h2 in range(2):
            jj = 2 * a + h2
            o[jj * CB : (jj + 1) * CB, :] = outT[
                H * h2 : H * (h2 + 1), a * CB : (a + 1) * CB
            ].T
    return o


def _ensure_axon_ntff_hook():
    """The agent image's antenv lacks axon_hooks; synthesize it so
    run_bass_kernel_spmd's trace path can find the NTFF profile hook."""
    import sys
    import types

    if "antenv.axon_hooks" in sys.modules:
        return
    try:
        import antenv

        mod = types.ModuleType("antenv.axon_hooks")
        mod._hook = None

        def set_axon_ntff_profile_hook(h):
            mod._hook = h

        def get_axon_ntff_profile_hook():
            return mod._hook

        mod.set_axon_ntff_profile_hook = set_axon_ntff_profile_hook
        mod.get_axon_ntff_profile_hook = get_axon_ntff_profile_hook
        sys.modules["antenv.axon_hooks"] = mod
        antenv.axon_hooks = mod

        from trn_agent_boot.trn_boot import _ntff_profile_via_ctypes

        hook = _ntff_profile_via_ctypes("/opt/axon/libaxon_pjrt.so")
        if hook is not None:
            mod._hook = hook
    except Exception as e:  # degrade to untraced run
        print(f"NTFF hook setup failed ({e}); tracing will be skipped")


def kernel(x, Wk, Wq, Wv, _trace=False, _trace_kwargs=None):
    if _trace:
        _ensure_axon_ntff_hook()
    in_maps = _host_inputs(x, Wk, Wq, Wv)
    nc = _build_program()
    res = bass_utils.run_bass_kernel_spmd(
        nc, in_maps, list(range(B)), trace=_trace, **(_trace_kwargs or {})
    )
    out = np.stack(
        [_unpack_out(res.results[b]["out"]) for b in range(B)], axis=0
    )
    if _trace:
        kernel.last_results = res
    return out.astype(np.float32)

